# revision 1
# baseline (speedup 1.0000x reference)
"""Trainium2 Bass kernel for nn_DecoderLayer (Mamba block + BitNet FFN).

Sharding: channel-parallel mamba (256 ch/core) -> AllReduce (xproj rows) ->
DVE tensor_tensor_scan over (d,n) lanes -> AllToAll (d-shard -> t-shard) ->
sequence-parallel out_proj + rmsnorm + BitNet FFN (host-prequantized ternary
weights, exact integer bf16 matmuls) -> each core emits its 256-token slice.
"""
import numpy as np
import ml_dtypes

try:
    import jax
    jax.config.update("jax_compilation_cache_dir", "/root/jaxcache")
    jax.config.update("jax_persistent_cache_min_compile_time_secs", 1.0)
except Exception:
    pass

import concourse.bass as bass
import concourse.mybir as mybir
import concourse.tile as tile
from concourse import bacc
from concourse.bass_utils import run_bass_kernel_spmd

BF16 = mybir.dt.bfloat16
F32 = mybir.dt.float32
F32R = mybir.dt.float32r
AF = mybir.ActivationFunctionType
OP = mybir.AluOpType

L, DM, DI, DS, DC, DTR, DFF = 2048, 1024, 2048, 16, 4, 64, 4096
EPS = 1e-6
NCORES = 8
DIC = DI // NCORES   # 256 channels per core
NDT = DIC // 128     # 2 d-tiles
LT = L // NCORES     # 256 tokens per core
NTT = LT // 128      # 2 token-tiles
MAGIC = 12582912.0   # 1.5*2^23: x+M-M == rint(x) for |x|<2^22

_NC_CACHE = {}


def _emit(nc, tc, ctx, g1, g2, dbg):
    import contextlib
    RG = [list(range(NCORES))]

    xT = nc.dram_tensor("xT", [DM, L], BF16, kind="ExternalInput")
    x_tok = nc.dram_tensor("x_tok", [LT, DM], F32, kind="ExternalInput")
    winT = nc.dram_tensor("winT", [DM, 2 * 128 * NDT], BF16, kind="ExternalInput")
    convw = nc.dram_tensor("convw", [DIC, DC], F32, kind="ExternalInput")
    convb = nc.dram_tensor("convb", [DIC, 1], F32, kind="ExternalInput")
    wxpT = nc.dram_tensor("wxpT", [DIC, 96], F32, kind="ExternalInput")
    wdtT = nc.dram_tensor("wdtT", [DTR, DIC], F32, kind="ExternalInput")
    bdt = nc.dram_tensor("bdt", [DIC, 1], F32, kind="ExternalInput")
    acol = nc.dram_tensor("acol", [DIC, DS], F32, kind="ExternalInput")
    dpv = nc.dram_tensor("dpv", [DIC, 1], F32, kind="ExternalInput")
    woutT = nc.dram_tensor("woutT", [DI, DM], BF16, kind="ExternalInput")
    n1w = nc.dram_tensor("n1w", [1, DM], F32, kind="ExternalInput")
    n2w = nc.dram_tensor("n2w", [1, DM], F32, kind="ExternalInput")
    w1qT = nc.dram_tensor("w1qT", [DM, DFF], BF16, kind="ExternalInput")
    w2qT = nc.dram_tensor("w2qT", [DFF, DM], BF16, kind="ExternalInput")
    out_t = nc.dram_tensor("out", [LT, DM], F32, kind="ExternalOutput")
    dbg_t = {}
    if dbg:
        dbg_t["dbg_u"] = nc.dram_tensor("dbg_u", [128, L], F32, kind="ExternalOutput")
        dbg_t["dbg_delta"] = nc.dram_tensor("dbg_delta", [128, L], F32, kind="ExternalOutput")
        dbg_t["dbg_dbl"] = nc.dram_tensor("dbg_dbl", [96, L], F32, kind="ExternalOutput")
        dbg_t["dbg_yhat"] = nc.dram_tensor("dbg_yhat", [128, L], F32, kind="ExternalOutput")
        dbg_t["dbg_x1"] = nc.dram_tensor("dbg_x1", [128, DM], F32, kind="ExternalOutput")
        dbg_t["dbg_f"] = nc.dram_tensor("dbg_f", [128, DFF], F32, kind="ExternalOutput")

    singles = ctx.enter_context(tc.tile_pool(name="singles", bufs=1))
    dram = ctx.enter_context(tc.tile_pool(name="dram", bufs=1, space="DRAM"))
    psA_stack = contextlib.ExitStack()
    psum_small = psA_stack.enter_context(
        tc.tile_pool(name="psA", bufs=3, space="PSUM"))
    act_stack = contextlib.ExitStack()
    actpool = act_stack.enter_context(tc.tile_pool(name="acts", bufs=1))

    # ---- small per-partition constants
    convw_sb, convb_sb, bdt_sb, acol_sb, dp_sb = [], [], [], [], []
    for dt in range(NDT):
        sl = slice(dt * 128, (dt + 1) * 128)
        t1 = singles.tile([128, DC], F32, name=f"cw{dt}")
        nc.sync.dma_start(t1[:, :], convw[sl, :])
        convw_sb.append(t1)
        t2 = singles.tile([128, 1], F32, name=f"cb{dt}")
        nc.sync.dma_start(t2[:, :], convb[sl, :])
        convb_sb.append(t2)
        t3 = singles.tile([128, 1], F32, name=f"bd{dt}")
        nc.sync.dma_start(t3[:, :], bdt[sl, :])
        bdt_sb.append(t3)
        t4 = singles.tile([128, DS], F32, name=f"ac{dt}")
        nc.sync.dma_start(t4[:, :], acol[sl, :])
        acol_sb.append(t4)
        t5 = singles.tile([128, 1], F32, name=f"dp{dt}")
        nc.sync.dma_start(t5[:, :], dpv[sl, :])
        dp_sb.append(t5)
    wxpT_raw = singles.tile([128, NDT, 96], F32)
    nc.sync.dma_start(wxpT_raw[:, :, :],
                      wxpT.rearrange("(k p) m -> p k m", p=128))
    wxpT_sb = singles.tile([128, NDT, 96], F32R)
    nc.vector.tensor_copy(wxpT_sb[:, :, :], wxpT_raw[:, :, :])
    wdtT_raw = singles.tile([DTR, DIC], F32)
    nc.sync.dma_start(wdtT_raw[:, :], wdtT[:, :])
    wdtT_sb = singles.tile([DTR, DIC], F32R)
    nc.vector.tensor_copy(wdtT_sb[:, :], wdtT_raw[:, :])
    ident_bf = singles.tile([128, 128], BF16)
    from concourse.masks import make_identity
    make_identity(nc, ident_bf[:, :])

    # ================= PHASE A: in_proj (channel-parallel) =================
    conv_stack = contextlib.ExitStack()
    convpool = conv_stack.enter_context(tc.tile_pool(name="convp", bufs=1))
    with tc.tile_pool(name="init", bufs=1) as init_pool:
        xT_sb = init_pool.tile([128, 8, L], BF16)
        nc.sync.dma_start(xT_sb[:, :, :], xT.rearrange("(k p) l -> p k l", p=128))
        winT_sb = init_pool.tile([128, 8, 2 * 128 * NDT], BF16)
        nc.sync.dma_start(winT_sb[:, :, :],
                          winT.rearrange("(k p) m -> p k m", p=128))

        u_pad, zs = [], []
        for dt in range(NDT):
            up = convpool.tile([128, L + 3], F32, name=f"upad{dt}")
            nc.vector.memset(up[:, 0:3], 0.0)
            u_pad.append(up)
            zs.append(actpool.tile([128, L], F32, name=f"zs{dt}"))

        # m-tiles: 0..NDT-1 are u chunks, NDT..2*NDT-1 are z chunks
        for mt in range(2 * NDT):
            for c in range(L // 512):
                ps = psum_small.tile([128, 512], F32, tag="psA")
                for k in range(8):
                    nc.tensor.matmul(
                        ps[:, :],
                        winT_sb[:, k, mt * 128:(mt + 1) * 128],
                        xT_sb[:, k, c * 512:(c + 1) * 512],
                        start=(k == 0), stop=(k == 7))
                if mt < NDT:
                    nc.scalar.copy(
                        u_pad[mt][:, 3 + c * 512: 3 + (c + 1) * 512], ps[:, :])
                else:
                    nc.scalar.activation(
                        zs[mt - NDT][:, c * 512:(c + 1) * 512], ps[:, :], AF.Silu)

    # ================= conv + silu =================
    u_act = []
    for dt in range(NDT):
        ca = convpool.tile([128, L], F32, name=f"cva{dt}", tag="cva")
        cb = convpool.tile([128, L], F32, name=f"cvb{dt}", tag="cvb")
        nc.vector.tensor_scalar_mul(ca[:, :], u_pad[dt][:, 0:L],
                                    convw_sb[dt][:, 0:1])
        nc.vector.scalar_tensor_tensor(
            cb[:, :], u_pad[dt][:, 1:L + 1], convw_sb[dt][:, 1:2], ca[:, :],
            op0=OP.mult, op1=OP.add)
        nc.vector.scalar_tensor_tensor(
            ca[:, :], u_pad[dt][:, 2:L + 2], convw_sb[dt][:, 2:3], cb[:, :],
            op0=OP.mult, op1=OP.add)
        nc.vector.scalar_tensor_tensor(
            cb[:, :], u_pad[dt][:, 3:L + 3], convw_sb[dt][:, 3:4], ca[:, :],
            op0=OP.mult, op1=OP.add)
        ua = actpool.tile([128, L], F32, name=f"uact{dt}")
        nc.scalar.activation(ua[:, :], cb[:, :], AF.Silu,
                             bias=convb_sb[dt][:, 0:1])
        u_act.append(ua)
    conv_stack.close()
    if dbg:
        nc.sync.dma_start(dbg_t["dbg_u"][:, :], u_act[0][:, :])

    # ================= xproj partial + AllReduce =================
    xp_stack = contextlib.ExitStack()
    xppool = xp_stack.enter_context(tc.tile_pool(name="xpp", bufs=1))
    u_r = []
    for dt in range(NDT):
        ur = xppool.tile([128, L], F32R, name=f"ur{dt}", tag=f"ur{dt}")
        nc.vector.tensor_copy(ur[:, :], u_act[dt][:, :])
        u_r.append(ur)
    dbl_loc = xppool.tile([96, L], F32)
    for c in range(L // 512):
        ps = psum_small.tile([96, 512], F32, tag="psA")
        for kt in range(NDT):
            nc.tensor.matmul(
                ps[:, :],
                wxpT_sb[:, kt, :],
                u_r[kt][:, c * 512:(c + 1) * 512],
                start=(kt == 0), stop=(kt == NDT - 1))
        nc.scalar.copy(dbl_loc[:, c * 512:(c + 1) * 512], ps[:, :])

    ar_i = dram.tile([96, L], F32)
    ar_o = dram.tile([96, L], F32, addr_space="Shared")
    nc.sync.dma_start(ar_i[:, :], dbl_loc[:, :])
    nc.gpsimd.collective_compute("AllReduce", OP.add, replica_groups=RG,
                                 ins=[ar_i.opt()], outs=[ar_o.opt()])
    dbl_sb = actpool.tile([96, L], F32)
    nc.sync.dma_start(dbl_sb[:, :], ar_o[:, :])
    if dbg:
        nc.sync.dma_start(dbg_t["dbg_dbl"][:, :], dbl_sb[:, :])

    # B/C rows -> bf16 bounce in DRAM for partition-replication
    bc_bf = actpool.tile([32, L], BF16)
    nc.vector.tensor_copy(bc_bf[:, :], dbl_sb[64:96, :])
    bcb = dram.tile([32, L], BF16)
    nc.sync.dma_start(bcb[:, :], bc_bf[:, :])

    # ================= delta = softplus(wdt @ dt + bdt) =================
    dt_r = xppool.tile([DTR, L], F32R)
    nc.vector.tensor_copy(dt_r[:, :], dbl_sb[0:DTR, :])
    delta = []
    for dt in range(NDT):
        dl = actpool.tile([128, L], F32, name=f"delta{dt}")
        for c in range(L // 512):
            ps = psum_small.tile([128, 512], F32, tag="psA")
            nc.tensor.matmul(
                ps[:, :],
                wdtT_sb[:, dt * 128:(dt + 1) * 128],
                dt_r[:, c * 512:(c + 1) * 512],
                start=True, stop=True)
            # exp(x + bdt) from PSUM, then ln(1+e) in-place later
            nc.scalar.activation(dl[:, c * 512:(c + 1) * 512], ps[:, :],
                                 AF.Exp, bias=bdt_sb[dt][:, 0:1])
        nc.scalar.activation(dl[:, :], dl[:, :], AF.Ln, bias=1.0)
        delta.append(dl)
    if dbg:
        nc.sync.dma_start(dbg_t["dbg_delta"][:, :], delta[0][:, :])

    xp_stack.close()
    # delta*u in bf16 for the scan input product
    du_bf = []
    for dt in range(NDT):
        db = actpool.tile([128, L], BF16, name=f"dubf{dt}")
        nc.vector.tensor_tensor(db[:, :], delta[dt][:, :], u_act[dt][:, :],
                                op=OP.mult)
        du_bf.append(db)

    # ================= scan over n (16 states) =================
    psA_stack.close()
    yps_stack = contextlib.ExitStack()
    y_ps_pool = yps_stack.enter_context(
        tc.tile_pool(name="yps", bufs=1, space="PSUM"))
    y_ps = [y_ps_pool.tile([128, L], F32, name=f"yps{dt}") for dt in range(NDT)]

    scanp = act_stack.enter_context(tc.tile_pool(name="scanp", bufs=2))
    repp = act_stack.enter_context(tc.tile_pool(name="repp", bufs=3))
    for n in range(DS):
        brep = repp.tile([128, L], BF16, name=f"brep{n}", tag="brep")
        b_src = bcb[n:n + 1, :]
        nc.sync.dma_start(brep[:, :], bass.AP(
            tensor=b_src.tensor, offset=b_src.offset,
            ap=[[0, 128]] + [list(p) for p in b_src.ap[1:]]))
        crep = repp.tile([128, L], BF16, name=f"crep{n}", tag="crep")
        c_src = bcb[16 + n:17 + n, :]
        nc.sync.dma_start(crep[:, :], bass.AP(
            tensor=c_src.tensor, offset=c_src.offset,
            ap=[[0, 128]] + [list(p) for p in c_src.ap[1:]]))
        for dt in range(NDT):
            dA = scanp.tile([128, L], BF16, name=f"dA{n}_{dt}", tag="dA")
            nc.scalar.activation(dA[:, :], delta[dt][:, :], AF.Exp,
                                 scale=acol_sb[dt][:, n:n + 1])
            dBu = scanp.tile([128, L], BF16, name=f"dBu{n}_{dt}", tag="dBu")
            nc.vector.tensor_tensor(dBu[:, :], du_bf[dt][:, :], brep[:, :],
                                    op=OP.mult)
            h = scanp.tile([128, L], BF16, name=f"h{n}_{dt}", tag="h")
            nc.vector.tensor_tensor_scan(h[:, :], dA[:, :], dBu[:, :], 0.0,
                                         OP.mult, OP.add)
            yt = scanp.tile([128, L], BF16, name=f"yt{n}_{dt}", tag="yt")
            nc.vector.tensor_tensor(yt[:, :], h[:, :], crep[:, :], op=OP.mult)
            for c in range(L // 512):
                nc.tensor.matmul(
                    y_ps[dt][:, c * 512:(c + 1) * 512],
                    ident_bf[:, :],
                    yt[:, c * 512:(c + 1) * 512],
                    start=(n == 0), stop=(n == DS - 1),
                    skip_group_check=True)

    # ================= gate: yhat = (y + Dp*u) * silu(z), A2A =================
    a2a_i = dram.tile([DI, LT], BF16)
    a2a_o = dram.tile([DI, LT], BF16)
    for dt in range(NDT):
        t1 = scanp.tile([128, L], F32, name=f"yg{dt}", tag="yg")
        nc.vector.scalar_tensor_tensor(
            t1[:, :], u_act[dt][:, :], dp_sb[dt][:, 0:1], y_ps[dt][:, :],
            op0=OP.mult, op1=OP.add)
        yh = scanp.tile([128, L], BF16, name=f"yhat{dt}", tag="yhat")
        nc.vector.tensor_tensor(yh[:, :], t1[:, :], zs[dt][:, :], op=OP.mult)
        if dbg and dt == 0:
            yh32 = scanp.tile([128, L], F32, name="yh32", tag="yh32")
            nc.vector.tensor_copy(yh32[:, :], yh[:, :])
            nc.sync.dma_start(dbg_t["dbg_yhat"][:, :], yh32[:, :])
        # scatter my 128-ch rows into (8 token-blocks x DIC) layout
        nc.sync.dma_start(
            a2a_i.rearrange("(j c) t -> c j t", c=DIC)[dt * 128:(dt + 1) * 128, :, :],
            yh.rearrange("c (j t) -> c j t", j=NCORES))
    nc.gpsimd.collective_compute("AllToAll", OP.bypass, replica_groups=RG,
                                 ins=[a2a_i.opt()], outs=[a2a_o.opt()])

    # ================= PHASE B (sequence-parallel, my LT tokens) ==========
    yps_stack.close()
    act_stack.close()
    bpool = ctx.enter_context(tc.tile_pool(name="bpool", bufs=1))
    psB = ctx.enter_context(tc.tile_pool(name="psB", bufs=2, space="PSUM"))

    x_tok_sb = bpool.tile([128, NTT, DM], F32)
    nc.sync.dma_start(x_tok_sb[:, :, :], x_tok.rearrange("(tt p) m -> p tt m", p=128))
    n1w_rep = bpool.tile([128, DM], F32)
    s1 = n1w[0:1, :]
    nc.sync.dma_start(n1w_rep[:, :], bass.AP(
        tensor=s1.tensor, offset=s1.offset,
        ap=[[0, 128]] + [list(p) for p in s1.ap[1:]]))
    n2w_rep = bpool.tile([128, DM], F32)
    s2 = n2w[0:1, :]
    nc.sync.dma_start(n2w_rep[:, :], bass.AP(
        tensor=s2.tensor, offset=s2.offset,
        ap=[[0, 128]] + [list(p) for p in s2.ap[1:]]))

    x1_l, scl1_l, xqT_l, fqT_l, scl2_l = [], [], [], [], []

    # ---- out_proj + rmsnorm1 + quant1 (weights freed after) ----
    with tc.tile_pool(name="oproj", bufs=1) as opool:
        yfull = opool.tile([128, DI // 128, LT], BF16)
        nc.sync.dma_start(yfull[:, :, :], a2a_o.rearrange("(k p) t -> p k t", p=128))
        woutT_sb = opool.tile([128, DI // 128, DM], BF16)
        nc.sync.dma_start(woutT_sb[:, :, :], woutT.rearrange("(k p) m -> p k m", p=128))
        for tt in range(NTT):
            hps = psB.tile([128, DM], F32, tag="hps")
            for c in range(DM // 512):
                for k in range(DI // 128):
                    nc.tensor.matmul(
                        hps[:, c * 512:(c + 1) * 512],
                        yfull[:, k, tt * 128:(tt + 1) * 128],
                        woutT_sb[:, k, c * 512:(c + 1) * 512],
                        start=(k == 0), stop=(k == DI // 128 - 1))
            s = bpool.tile([128, DM], F32, name=f"s{tt}", tag=f"s{tt}")
            nc.vector.tensor_tensor(s[:, :], x_tok_sb[:, tt, :], hps[:, :], op=OP.add)
            sq = bpool.tile([128, DM], F32, name=f"sq{tt}", tag="sq")
            ssum = bpool.tile([128, 1], F32, name=f"ssum{tt}", tag="ssum")
            nc.scalar.activation(sq[:, :], s[:, :], AF.Square, accum_out=ssum[:, 0:1])
            v = bpool.tile([128, 1], F32, name=f"v{tt}", tag=f"v{tt}")
            nc.vector.tensor_scalar(v[:, :], ssum[:, :], 1.0 / DM, EPS,
                                    op0=OP.mult, op1=OP.add)
            nc.scalar.activation(v[:, :], v[:, :], AF.Ln)
            nc.scalar.activation(v[:, :], v[:, :], AF.Exp, scale=-0.5)
            x1 = bpool.tile([128, DM], F32, name=f"x1_{tt}", tag=f"x1_{tt}")
            nc.vector.scalar_tensor_tensor(x1[:, :], s[:, :], v[:, 0:1],
                                           n1w_rep[:, :], op0=OP.mult, op1=OP.mult)
            x1_l.append(x1)
            if dbg and tt == 0:
                nc.sync.dma_start(dbg_t["dbg_x1"][:, :], x1[:, :])
            amax = bpool.tile([128, 1], F32, name=f"am{tt}", tag="am")
            nc.vector.tensor_reduce(amax[:, :], x1[:, :], axis=mybir.AxisListType.X,
                                    op=OP.max, apply_absolute_value=True)
            nc.vector.tensor_scalar(amax[:, :], amax[:, :], 1e-5, None, op0=OP.max)
            sc = bpool.tile([128, 1], F32, name=f"sc{tt}", tag="sc")
            nc.vector.reciprocal(sc[:, :], amax[:, :])
            nc.vector.tensor_scalar(sc[:, :], sc[:, :], 127.0, None, op0=OP.mult)
            scl1 = bpool.tile([128, 1], F32, name=f"scl1_{tt}", tag=f"scl1_{tt}")
            nc.vector.tensor_scalar(scl1[:, :], amax[:, :], g1 / 127.0, None,
                                    op0=OP.mult)
            scl1_l.append(scl1)
            q = bpool.tile([128, DM], F32, name=f"q{tt}", tag="q")
            nc.vector.tensor_scalar(q[:, :], x1[:, :], sc[:, 0:1], None, op0=OP.mult)
            nc.vector.tensor_scalar(q[:, :], q[:, :], MAGIC, MAGIC,
                                    op0=OP.add, op1=OP.subtract)
            xq = bpool.tile([128, DM], BF16, name=f"xq{tt}", tag="xq")
            nc.vector.tensor_scalar(xq[:, :], q[:, :], 127.0, -128.0,
                                    op0=OP.min, op1=OP.max)
            xqT = bpool.tile([128, DM // 128, 128], BF16, name=f"xqT{tt}",
                             tag=f"xqT{tt}")
            nc.sync.dma_start_transpose(xqT[:, :, :], xq[:, :])
            xqT_l.append(xqT)

    # ---- FFN mm1 + gelu + quant2 (w1 freed after) ----
    with tc.tile_pool(name="ffn1", bufs=1) as f1pool:
        w1qT_sb = f1pool.tile([128, 8, DFF], BF16)
        nc.sync.dma_start(w1qT_sb[:, :, :], w1qT.rearrange("(k p) j -> p k j", p=128))
        for tt in range(NTT):
            f_sb = f1pool.tile([128, DFF], F32, name=f"f{tt}", tag="f")
            for jc in range(DFF // 512):
                fps = psB.tile([128, 512], F32, tag="fps")
                for k in range(DM // 128):
                    nc.tensor.matmul(
                        fps[:, :], xqT_l[tt][:, k, :],
                        w1qT_sb[:, k, jc * 512:(jc + 1) * 512],
                        start=(k == 0), stop=(k == DM // 128 - 1))
                nc.scalar.activation(f_sb[:, jc * 512:(jc + 1) * 512], fps[:, :],
                                     AF.Gelu_apprx_tanh, scale=scl1_l[tt][:, 0:1])
            if dbg and tt == 0:
                nc.sync.dma_start(dbg_t["dbg_f"][:, :], f_sb[:, :])
            amax2 = bpool.tile([128, 1], F32, name=f"am2{tt}", tag="am2")
            nc.vector.tensor_reduce(amax2[:, :], f_sb[:, :], axis=mybir.AxisListType.X,
                                    op=OP.max, apply_absolute_value=True)
            nc.vector.tensor_scalar(amax2[:, :], amax2[:, :], 1e-5, None, op0=OP.max)
            sc2 = bpool.tile([128, 1], F32, name=f"sc2{tt}", tag="sc2")
            nc.vector.reciprocal(sc2[:, :], amax2[:, :])
            nc.vector.tensor_scalar(sc2[:, :], sc2[:, :], 127.0, None, op0=OP.mult)
            scl2 = bpool.tile([128, 1], F32, name=f"scl2_{tt}", tag=f"scl2_{tt}")
            nc.vector.tensor_scalar(scl2[:, :], amax2[:, :], g2 / 127.0, None,
                                    op0=OP.mult)
            scl2_l.append(scl2)
            q2 = f1pool.tile([128, DFF], F32, name=f"q2{tt}", tag="q2")
            nc.vector.tensor_scalar(q2[:, :], f_sb[:, :], sc2[:, 0:1], None,
                                    op0=OP.mult)
            nc.vector.tensor_scalar(q2[:, :], q2[:, :], MAGIC, MAGIC,
                                    op0=OP.add, op1=OP.subtract)
            fq = f1pool.tile([128, DFF], BF16, name=f"fq{tt}", tag="fq")
            nc.vector.tensor_scalar(fq[:, :], q2[:, :], 127.0, -128.0,
                                    op0=OP.min, op1=OP.max)
            fqT = bpool.tile([128, DFF // 128, 128], BF16, name=f"fqT{tt}",
                             tag=f"fqT{tt}")
            nc.sync.dma_start_transpose(fqT[:, :, :], fq[:, :])
            fqT_l.append(fqT)

    # ---- FFN mm2 + residual + rmsnorm2 ----
    with tc.tile_pool(name="ffn2", bufs=1) as f2pool:
        w2qT_sb = f2pool.tile([128, DFF // 128, DM], BF16)
        nc.sync.dma_start(w2qT_sb[:, :, :], w2qT.rearrange("(k p) m -> p k m", p=128))
        for tt in range(NTT):
            o2 = f2pool.tile([128, DM], F32, name=f"o2{tt}", tag="o2")
            for mc in range(DM // 512):
                ops_ = psB.tile([128, 512], F32, tag="ops")
                for k in range(DFF // 128):
                    nc.tensor.matmul(
                        ops_[:, :], fqT_l[tt][:, k, :],
                        w2qT_sb[:, k, mc * 512:(mc + 1) * 512],
                        start=(k == 0), stop=(k == DFF // 128 - 1))
                nc.vector.scalar_tensor_tensor(
                    o2[:, mc * 512:(mc + 1) * 512], ops_[:, :], scl2_l[tt][:, 0:1],
                    x1_l[tt][:, mc * 512:(mc + 1) * 512], op0=OP.mult, op1=OP.add)
            sq2 = f2pool.tile([128, DM], F32, name=f"sq2{tt}", tag="sq2")
            ssum2 = f2pool.tile([128, 1], F32, name=f"ssum2{tt}", tag="ssum2")
            nc.scalar.activation(sq2[:, :], o2[:, :], AF.Square,
                                 accum_out=ssum2[:, 0:1])
            v2 = f2pool.tile([128, 1], F32, name=f"v2{tt}", tag=f"v2{tt}")
            nc.vector.tensor_scalar(v2[:, :], ssum2[:, :], 1.0 / DM, EPS,
                                    op0=OP.mult, op1=OP.add)
            nc.scalar.activation(v2[:, :], v2[:, :], AF.Ln)
            nc.scalar.activation(v2[:, :], v2[:, :], AF.Exp, scale=-0.5)
            ot = f2pool.tile([128, DM], F32, name=f"ot{tt}", tag="ot")
            nc.vector.scalar_tensor_tensor(ot[:, :], o2[:, :], v2[:, 0:1],
                                           n2w_rep[:, :], op0=OP.mult, op1=OP.mult)
            nc.sync.dma_start(out_t[tt * 128:(tt + 1) * 128, :], ot[:, :])


def build_nc(g1, g2, dbg=False):
    from contextlib import ExitStack
    nc = bacc.Bacc("TRN2", target_bir_lowering=False, debug=False,
                   num_devices=NCORES)
    with ExitStack() as ctx:
        tc = ctx.enter_context(tile.TileContext(nc))
        _emit(nc, tc, ctx, g1, g2, dbg)
    nc.compile()
    return nc


def host_prep(inputs):
    bf = ml_dtypes.bfloat16
    x = np.asarray(inputs["x"], np.float32)
    x2d = x.reshape(L, DM)
    w_in = np.asarray(inputs["w_in"], np.float32)
    conv_w = np.asarray(inputs["conv_w"], np.float32)
    conv_b = np.asarray(inputs["conv_b"], np.float32)
    w_xproj = np.asarray(inputs["w_xproj"], np.float32)
    w_dt = np.asarray(inputs["w_dt"], np.float32)
    b_dt = np.asarray(inputs["b_dt"], np.float32)
    A_log = np.asarray(inputs["A_log"], np.float32)
    Dp = np.asarray(inputs["Dp"], np.float32)
    w_out = np.asarray(inputs["w_out"], np.float32)
    n1 = np.asarray(inputs["norm1_w"], np.float32)
    n2 = np.asarray(inputs["norm2_w"], np.float32)
    w1 = np.asarray(inputs["ffn_w1"], np.float32)
    w2 = np.asarray(inputs["ffn_w2"], np.float32)
    b1 = np.asarray(inputs["ffn_b1"], np.float32)
    b2 = np.asarray(inputs["ffn_b2"], np.float32)
    assert np.all(b1 == 0.0) and np.all(b2 == 0.0), "nonzero ffn bias unsupported"

    g1 = float(np.maximum(np.mean(np.abs(w1), dtype=np.float32), 1e-5))
    g2 = float(np.maximum(np.mean(np.abs(w2), dtype=np.float32), 1e-5))
    w1q = np.clip(np.rint(w1 / g1), -1.0, 1.0).astype(np.float32)
    w2q = np.clip(np.rint(w2 / g2), -1.0, 1.0).astype(np.float32)

    xT_bf = np.ascontiguousarray(x2d.T).astype(bf)
    woutT_bf = np.ascontiguousarray(w_out.T).astype(bf)
    w1qT_bf = np.ascontiguousarray(w1q.T).astype(bf)
    w2qT_bf = np.ascontiguousarray(w2q.T).astype(bf)
    n1r = np.ascontiguousarray(n1.reshape(1, DM))
    n2r = np.ascontiguousarray(n2.reshape(1, DM))
    A = -np.exp(A_log)

    in_maps = []
    for c in range(NCORES):
        ch = slice(c * DIC, (c + 1) * DIC)
        w_sel = np.concatenate([w_in[c * DIC:(c + 1) * DIC],
                                w_in[DI + c * DIC:DI + (c + 1) * DIC]], axis=0)
        in_maps.append({
            "xT": xT_bf,
            "x_tok": np.ascontiguousarray(x2d[c * LT:(c + 1) * LT]),
            "winT": np.ascontiguousarray(w_sel.T).astype(bf),
            "convw": np.ascontiguousarray(conv_w[ch, 0, :]),
            "convb": np.ascontiguousarray(conv_b[ch].reshape(-1, 1)),
            "wxpT": np.ascontiguousarray(w_xproj[:, ch].T),
            "wdtT": np.ascontiguousarray(w_dt[ch, :].T),
            "bdt": np.ascontiguousarray(b_dt[ch].reshape(-1, 1)),
            "acol": np.ascontiguousarray(A[ch, :]),
            "dpv": np.ascontiguousarray(Dp[ch].reshape(-1, 1)),
            "woutT": woutT_bf,
            "n1w": n1r,
            "n2w": n2r,
            "w1qT": w1qT_bf,
            "w2qT": w2qT_bf,
        })
    return in_maps, g1, g2


def kernel(**inputs) -> np.ndarray:
    in_maps, g1, g2 = host_prep(inputs)
    key = (round(g1, 10), round(g2, 10))
    if key not in _NC_CACHE:
        _NC_CACHE[key] = build_nc(g1, g2)
    nc = _NC_CACHE[key]
    res = run_bass_kernel_spmd(nc, in_maps, core_ids=list(range(NCORES)))
    out = np.concatenate([res.results[c]["out"] for c in range(NCORES)], axis=0)
    return np.ascontiguousarray(out.reshape(1, L, DM).astype(np.float32))



# revision 2
# speedup vs baseline: 50.8178x; 50.8178x over previous
"""Trainium2 Bass kernel v2 for nn_DecoderLayer (Mamba block + BitNet FFN).

Sharding: channel-parallel mamba (256 ch/core) -> AllReduce (xproj rows) ->
DVE tensor_tensor_scan over (d,n) lanes -> AllToAll (d-shard -> t-shard) ->
sequence-parallel out_proj + rmsnorm + BitNet FFN (fp8 ternary weights) ->
each core emits its 256-token slice.

v2: fp8e4 FFN weights (halved HBM/tunnel bytes), w1 prefetched at t0 and
wout/w2 loads overlapped with scan/A2A, bf16 datapath for 2x DVE modes,
B/C replication split across both HWDGE rings, native Softplus, PE
transposes instead of DMA transposes, chunked xT streaming.
"""
import numpy as np
import ml_dtypes

try:
    import jax
    jax.config.update("jax_compilation_cache_dir", "/root/jaxcache")
    jax.config.update("jax_persistent_cache_min_compile_time_secs", 1.0)
except Exception:
    pass

import concourse.bass as bass
import concourse.mybir as mybir
import concourse.tile as tile
from concourse import bacc
from concourse.bass_utils import run_bass_kernel_spmd

BF16 = mybir.dt.bfloat16
F32 = mybir.dt.float32
FP8 = mybir.dt.float8e4
AF = mybir.ActivationFunctionType
OP = mybir.AluOpType

L, DM, DI, DS, DC, DTR, DFF = 2048, 1024, 2048, 16, 4, 64, 4096
EPS = 1e-6
NCORES = 8
DIC = DI // NCORES   # 256 channels per core
NDT = DIC // 128     # 2 d-tiles
LT = L // NCORES     # 256 tokens per core
NTT = LT // 128      # 2 token-tiles
MAGIC = 12582912.0   # 1.5*2^23: x+M-M == rint(x) for |x|<2^22 (f32)
MAGIC_BF = 384.0     # 1.5*2^8: bf16 magic for ints in [-128,127]

_NC_CACHE = {}


def _rep_ap(src):
    """Broadcast a 1-row DRAM slice across 128 partitions."""
    return bass.AP(tensor=src.tensor, offset=src.offset,
                   ap=[[0, 128]] + [list(p) for p in src.ap[1:]])


def _emit(nc, tc, ctx, g1, g2, dbg):
    import contextlib
    RG = [list(range(NCORES))]

    xT = nc.dram_tensor("xT", [DM, L], BF16, kind="ExternalInput")
    x_tok = nc.dram_tensor("x_tok", [LT, DM], F32, kind="ExternalInput")
    winT = nc.dram_tensor("winT", [DM, 2 * 128 * NDT], BF16, kind="ExternalInput")
    convw = nc.dram_tensor("convw", [DIC, DC], F32, kind="ExternalInput")
    convb = nc.dram_tensor("convb", [DIC, 1], F32, kind="ExternalInput")
    wxpT = nc.dram_tensor("wxpT", [DIC, 96], BF16, kind="ExternalInput")
    wdtT = nc.dram_tensor("wdtT", [DTR, DIC], BF16, kind="ExternalInput")
    bdt = nc.dram_tensor("bdt", [DIC, 1], F32, kind="ExternalInput")
    acol = nc.dram_tensor("acol", [DIC, DS], F32, kind="ExternalInput")
    dpv = nc.dram_tensor("dpv", [DIC, 1], F32, kind="ExternalInput")
    woutT = nc.dram_tensor("woutT", [DI, DM], BF16, kind="ExternalInput")
    n1w = nc.dram_tensor("n1w", [1, DM], F32, kind="ExternalInput")
    n2w = nc.dram_tensor("n2w", [1, DM], F32, kind="ExternalInput")
    w1qT = nc.dram_tensor("w1qT", [DM, DFF], FP8, kind="ExternalInput")
    w2qT = nc.dram_tensor("w2qT", [DFF, DM], FP8, kind="ExternalInput")
    out_t = nc.dram_tensor("out", [LT, DM], F32, kind="ExternalOutput")
    dbg_t = {}
    if dbg:
        dbg_t["dbg_u"] = nc.dram_tensor("dbg_u", [128, L], F32, kind="ExternalOutput")
        dbg_t["dbg_delta"] = nc.dram_tensor("dbg_delta", [128, L], F32, kind="ExternalOutput")
        dbg_t["dbg_dbl"] = nc.dram_tensor("dbg_dbl", [96, L], F32, kind="ExternalOutput")
        dbg_t["dbg_yhat"] = nc.dram_tensor("dbg_yhat", [128, L], F32, kind="ExternalOutput")
        dbg_t["dbg_x1"] = nc.dram_tensor("dbg_x1", [128, DM], F32, kind="ExternalOutput")
        dbg_t["dbg_f"] = nc.dram_tensor("dbg_f", [128, DFF], F32, kind="ExternalOutput")

    singles = ctx.enter_context(tc.tile_pool(name="singles", bufs=1))
    dram = ctx.enter_context(tc.tile_pool(name="dram", bufs=1, space="DRAM"))
    psA_stack = contextlib.ExitStack()
    psum_small = psA_stack.enter_context(
        tc.tile_pool(name="psA", bufs=3, space="PSUM"))
    # w1-only prefetch pool (w2 gets its own late pool)
    ffnwpool = ctx.enter_context(tc.tile_pool(name="ffnwp", bufs=1))
    bpool = ctx.enter_context(tc.tile_pool(name="bpool", bufs=1))
    act_stack = contextlib.ExitStack()
    actpool = act_stack.enter_context(tc.tile_pool(name="acts", bufs=1))
    xp_stack = contextlib.ExitStack()
    xppool = xp_stack.enter_context(tc.tile_pool(name="xpp", bufs=1))
    conv_stack = contextlib.ExitStack()
    convpool = conv_stack.enter_context(tc.tile_pool(name="convp", bufs=2))
    init_stack = contextlib.ExitStack()
    init_pool = init_stack.enter_context(tc.tile_pool(name="init", bufs=2))

    # ---- winT first on SP ring so in_proj can start immediately ----
    winT_sb = init_pool.tile([128, 8, 2 * 128 * NDT], BF16, name="winT")
    nc.scalar.dma_start(winT_sb[:, :, :],
                        winT.rearrange("(k p) m -> p k m", p=128))
    # ---- prefetches on Act ring (overlap Phase A) ----
    w1qT_sb = ffnwpool.tile([128, DM // 128, DFF], FP8)
    nc.scalar.dma_start(w1qT_sb[:, :, :], w1qT.rearrange("(k p) j -> p k j", p=128))
    x_tok_sb = bpool.tile([128, NTT, DM], F32)
    nc.scalar.dma_start(x_tok_sb[:, :, :], x_tok.rearrange("(tt p) m -> p tt m", p=128))
    n1w_rep = bpool.tile([128, DM], F32)
    nc.scalar.dma_start(n1w_rep[:, :], _rep_ap(n1w[0:1, :]))
    n2w_rep = bpool.tile([128, DM], F32)
    nc.scalar.dma_start(n2w_rep[:, :], _rep_ap(n2w[0:1, :]))

    # ---- small per-partition constants (SP ring, after winT) ----
    convw_sb, convb_sb, bdt_sb, acol_sb, dp_sb = [], [], [], [], []
    for dt in range(NDT):
        sl = slice(dt * 128, (dt + 1) * 128)
        t1 = singles.tile([128, DC], F32, name=f"cw{dt}")
        nc.sync.dma_start(t1[:, :], convw[sl, :])
        convw_sb.append(t1)
        t2 = singles.tile([128, 1], F32, name=f"cb{dt}")
        nc.sync.dma_start(t2[:, :], convb[sl, :])
        convb_sb.append(t2)
        t3 = singles.tile([128, 1], F32, name=f"bd{dt}")
        nc.sync.dma_start(t3[:, :], bdt[sl, :])
        bdt_sb.append(t3)
        t4 = singles.tile([128, DS], F32, name=f"ac{dt}")
        nc.sync.dma_start(t4[:, :], acol[sl, :])
        acol_sb.append(t4)
        t5 = singles.tile([128, 1], F32, name=f"dp{dt}")
        nc.sync.dma_start(t5[:, :], dpv[sl, :])
        dp_sb.append(t5)
    wxpT_sb = singles.tile([128, NDT, 96], BF16)
    nc.sync.dma_start(wxpT_sb[:, :, :], wxpT.rearrange("(k p) m -> p k m", p=128))
    wdtT_sb = singles.tile([DTR, DIC], BF16)
    nc.sync.dma_start(wdtT_sb[:, :], wdtT[:, :])
    ident_bf = singles.tile([128, 128], BF16)
    from concourse.masks import make_identity
    make_identity(nc, ident_bf[:, :])

    # ============ PHASE A: in_proj + conv + xproj, chunk-pipelined ============
    dbl_loc = xppool.tile([96, L], BF16)
    xTr = xT.rearrange("(k p) l -> p k l", p=128)
    ar_i = dram.tile([96, L], BF16)
    u_pad, zs, u_act = [], [], []
    for dt in range(NDT):
        up = convpool.tile([128, L + 3], BF16, name=f"upad{dt}")
        nc.vector.memset(up[:, 0:3], 0.0)
        u_pad.append(up)
        zs.append(actpool.tile([128, L], BF16, name=f"zs{dt}"))
        u_act.append(actpool.tile([128, L], BF16, name=f"uact{dt}"))

    for c in range(L // 512):
        lo, hi = c * 512, (c + 1) * 512
        xT_c = init_pool.tile([128, 8, 512], BF16, tag="xTc")
        nc.sync.dma_start(xT_c[:, :, :], xTr[:, :, lo:hi])
        # m-tiles: 0..NDT-1 are u chunks, NDT..2*NDT-1 are z chunks
        for mt in range(2 * NDT):
            ps = psum_small.tile([128, 512], F32, tag="psA")
            for k in range(8):
                nc.tensor.matmul(
                    ps[:, :],
                    winT_sb[:, k, mt * 128:(mt + 1) * 128],
                    xT_c[:, k, :],
                    start=(k == 0), stop=(k == 7))
            if mt < NDT:
                nc.scalar.copy(u_pad[mt][:, 3 + lo: 3 + hi], ps[:, :])
            else:
                nc.scalar.activation(
                    zs[mt - NDT][:, lo:hi], ps[:, :], AF.Silu)
        # conv chunk (u_pad up to c*512+515 ready; 3-pad covers chunk c-1)
        for dt in range(NDT):
            ca = convpool.tile([128, 512], F32, tag="cva")
            cb = convpool.tile([128, 512], F32, tag="cvb")
            nc.vector.tensor_scalar_mul(ca[:, :], u_pad[dt][:, lo:hi],
                                        convw_sb[dt][:, 0:1])
            nc.vector.scalar_tensor_tensor(
                cb[:, :], u_pad[dt][:, lo + 1:hi + 1], convw_sb[dt][:, 1:2],
                ca[:, :], op0=OP.mult, op1=OP.add)
            nc.vector.scalar_tensor_tensor(
                ca[:, :], u_pad[dt][:, lo + 2:hi + 2], convw_sb[dt][:, 2:3],
                cb[:, :], op0=OP.mult, op1=OP.add)
            nc.vector.scalar_tensor_tensor(
                cb[:, :], u_pad[dt][:, lo + 3:hi + 3], convw_sb[dt][:, 3:4],
                ca[:, :], op0=OP.mult, op1=OP.add)
            nc.scalar.activation(u_act[dt][:, lo:hi], cb[:, :], AF.Silu,
                                 bias=convb_sb[dt][:, 0:1])
        # xproj partial for this chunk + AR staging
        ps96 = psum_small.tile([96, 512], F32, tag="ps96")
        for kt in range(NDT):
            nc.tensor.matmul(
                ps96[:, :],
                wxpT_sb[:, kt, :],
                u_act[kt][:, lo:hi],
                start=(kt == 0), stop=(kt == NDT - 1))
        nc.scalar.copy(dbl_loc[:, lo:hi], ps96[:, :])
        nc.sync.dma_start(ar_i[:, lo:hi], dbl_loc[:, lo:hi])
    init_stack.close()
    conv_stack.close()
    if dbg:
        u32 = actpool.tile([128, L], F32, name="u32dbg")
        nc.vector.tensor_copy(u32[:, :], u_act[0][:, :])
        nc.sync.dma_start(dbg_t["dbg_u"][:, :], u32[:, :])

    # ================= AllReduce (bf16) =================
    ar_o = dram.tile([96, L], BF16, addr_space="Shared")
    nc.gpsimd.collective_compute("AllReduce", OP.add, replica_groups=RG,
                                 ins=[ar_i.opt()], outs=[ar_o.opt()])
    # dt rows for the delta matmul; B/C rows are read straight from ar_o
    dt_bf = xppool.tile([DTR, L], BF16)
    nc.sync.dma_start(dt_bf[:, :], ar_o[0:DTR, :])
    if dbg:
        dbl_sb = xppool.tile([96, L], F32)
        nc.sync.dma_start(dbl_sb[:, :], ar_o[:, :])
        nc.sync.dma_start(dbg_t["dbg_dbl"][:, :], dbl_sb[:, :])
    bcb = ar_o[64:96, :]

    # ================= delta = softplus(wdt @ dt + bdt), bf16 ============
    delta = []
    for dt in range(NDT):
        dl = actpool.tile([128, L], BF16, name=f"delta{dt}")
        for c in range(L // 512):
            ps = psum_small.tile([128, 512], F32, tag="psA")
            nc.tensor.matmul(
                ps[:, :],
                wdtT_sb[:, dt * 128:(dt + 1) * 128],
                dt_bf[:, c * 512:(c + 1) * 512],
                start=True, stop=True)
            nc.scalar.activation(dl[:, c * 512:(c + 1) * 512], ps[:, :],
                                 AF.Exp, bias=bdt_sb[dt][:, 0:1])
        nc.scalar.activation(dl[:, :], dl[:, :], AF.Ln, bias=1.0)
        delta.append(dl)
    if dbg:
        d32 = actpool.tile([128, L], F32, name="d32dbg")
        nc.vector.tensor_copy(d32[:, :], delta[0][:, :])
        nc.sync.dma_start(dbg_t["dbg_delta"][:, :], d32[:, :])

    xp_stack.close()
    # delta*u in bf16 for the scan input product (both bf16 -> 2x DVE)
    du_bf = []
    for dt in range(NDT):
        db = actpool.tile([128, L], BF16, name=f"dubf{dt}")
        nc.vector.tensor_tensor(db[:, :], delta[dt][:, :], u_act[dt][:, :],
                                op=OP.mult)
        du_bf.append(db)

    # ================= scan over n (16 states) =================
    psA_stack.close()
    yps_stack = contextlib.ExitStack()
    y_ps_pool = yps_stack.enter_context(
        tc.tile_pool(name="yps", bufs=1, space="PSUM"))
    y_ps = [y_ps_pool.tile([128, L], F32, name=f"yps{dt}") for dt in range(NDT)]

    scanp = act_stack.enter_context(tc.tile_pool(name="scanp", bufs=3))
    repp = act_stack.enter_context(tc.tile_pool(name="repp", bufs=3))
    a2a_i = [dram.tile([DI // 2, LT], BF16, name=f"a2ai{d}") for d in range(NDT)]
    a2a_o = [dram.tile([DI // 2, LT], BF16, name=f"a2ao{d}") for d in range(NDT)]
    for dt in range(NDT):
        for n in range(DS):
            brep = repp.tile([128, L], BF16, name=f"brep{dt}_{n}", tag="brep")
            nc.sync.dma_start(brep[:, :], _rep_ap(bcb[n:n + 1, :]))
            crep = repp.tile([128, L], BF16, name=f"crep{dt}_{n}", tag="crep")
            nc.scalar.dma_start(crep[:, :], _rep_ap(bcb[16 + n:17 + n, :]))
            dA = scanp.tile([128, L], BF16, name=f"dA{n}_{dt}", tag="dA")
            nc.scalar.activation(dA[:, :], delta[dt][:, :], AF.Exp,
                                 scale=acol_sb[dt][:, n:n + 1])
            dBu = scanp.tile([128, L], BF16, name=f"dBu{n}_{dt}", tag="dBu")
            nc.vector.tensor_tensor(dBu[:, :], du_bf[dt][:, :], brep[:, :],
                                    op=OP.mult)
            h = scanp.tile([128, L], BF16, name=f"h{n}_{dt}", tag="h")
            nc.vector.tensor_tensor_scan(h[:, :], dA[:, :], dBu[:, :], 0.0,
                                         OP.mult, OP.add)
            yt = scanp.tile([128, L], BF16, name=f"yt{n}_{dt}", tag="yt")
            eng_tt = nc.gpsimd if n % 2 == 1 else nc.vector
            eng_tt.tensor_tensor(yt[:, :], h[:, :], crep[:, :], op=OP.mult)
            for c in range(L // 512):
                nc.tensor.matmul(
                    y_ps[dt][:, c * 512:(c + 1) * 512],
                    ident_bf[:, :],
                    yt[:, c * 512:(c + 1) * 512],
                    start=(n == 0), stop=(n == DS - 1),
                    skip_group_check=True)
        # gate for this dt, then its half-A2A (overlaps the other dt's scan)
        t1 = scanp.tile([128, L], BF16, name=f"yg{dt}", tag="dA")
        nc.vector.scalar_tensor_tensor(
            t1[:, :], u_act[dt][:, :], dp_sb[dt][:, 0:1], y_ps[dt][:, :],
            op0=OP.mult, op1=OP.add)
        yh = scanp.tile([128, L], BF16, name=f"yhat{dt}", tag="dBu")
        nc.vector.tensor_tensor(yh[:, :], t1[:, :], zs[dt][:, :], op=OP.mult)
        if dbg and dt == 0:
            yh32 = scanp.tile([128, L], F32, name="yh32", tag="yh32")
            nc.vector.tensor_copy(yh32[:, :], yh[:, :])
            nc.sync.dma_start(dbg_t["dbg_yhat"][:, :], yh32[:, :])
        # scatter my 128-ch rows into (8 token-blocks x 128) layout
        nc.sync.dma_start(
            a2a_i[dt].rearrange("(j c) t -> c j t", c=128)[:, :, :],
            yh.rearrange("c (j t) -> c j t", j=NCORES))
        nc.gpsimd.collective_compute("AllToAll", OP.bypass, replica_groups=RG,
                                     ins=[a2a_i[dt].opt()],
                                     outs=[a2a_o[dt].opt()])

    # ================= PHASE B (sequence-parallel, my LT tokens) ==========
    yps_stack.close()
    act_stack.close()
    psB = ctx.enter_context(tc.tile_pool(name="psB", bufs=2, space="PSUM"))
    psT = ctx.enter_context(tc.tile_pool(name="psT", bufs=2, space="PSUM"))
    scr = ctx.enter_context(tc.tile_pool(name="scr", bufs=1))
    bpB = ctx.enter_context(tc.tile_pool(name="bpB", bufs=1))
    w2pool = ctx.enter_context(tc.tile_pool(name="w2p", bufs=1))
    # wout + w2 loads: start as soon as scan-era SBUF frees; overlap A2A wait
    wout_stack = contextlib.ExitStack()
    woutpool = wout_stack.enter_context(tc.tile_pool(name="woutp", bufs=1))
    woutT_sb = woutpool.tile([128, DI // 128, DM], BF16)
    nc.sync.dma_start(woutT_sb[:, :, :], woutT.rearrange("(k p) m -> p k m", p=128))
    w2qT_sb = w2pool.tile([128, DFF // 128, DM], FP8)
    nc.scalar.dma_start(w2qT_sb[:, :, :], w2qT.rearrange("(k p) m -> p k m", p=128))

    x1_l, scl1_l, xqT_l, fqT_l, scl2_l = [], [], [], [], []

    # ---- out_proj + rmsnorm1 + quant1 ----
    with tc.tile_pool(name="oproj", bufs=1) as opool:
        yfull = opool.tile([128, NDT, NCORES, LT], BF16)
        for d in range(NDT):
            nc.sync.dma_start(yfull[:, d, :, :],
                              a2a_o[d].rearrange("(j p) t -> p j t", p=128))
        for tt in range(NTT):
            hps = psB.tile([128, DM], F32, tag="hps")
            for c in range(DM // 512):
                kk = 0
                for d in range(NDT):
                    for j in range(NCORES):
                        nc.tensor.matmul(
                            hps[:, c * 512:(c + 1) * 512],
                            yfull[:, d, j, tt * 128:(tt + 1) * 128],
                            woutT_sb[:, j * NDT + d, c * 512:(c + 1) * 512],
                            start=(kk == 0), stop=(kk == DI // 128 - 1))
                        kk += 1
            s = scr.tile([128, DM], F32, name=f"s{tt}", tag="sscr")
            nc.vector.tensor_tensor(s[:, :], x_tok_sb[:, tt, :], hps[:, :], op=OP.add)
            sq = scr.tile([128, DM], F32, name=f"sq{tt}", tag="sqscr")
            ssum = bpB.tile([128, 1], F32, name=f"ssum{tt}", tag="ssum")
            nc.scalar.activation(sq[:, :], s[:, :], AF.Square, accum_out=ssum[:, 0:1])
            v = bpB.tile([128, 1], F32, name=f"v{tt}", tag=f"v{tt}")
            nc.vector.tensor_scalar(v[:, :], ssum[:, :], 1.0 / DM, EPS,
                                    op0=OP.mult, op1=OP.add)
            nc.scalar.activation(v[:, :], v[:, :], AF.Sqrt)
            nc.vector.reciprocal(v[:, :], v[:, :])
            x1 = bpB.tile([128, DM], F32, name=f"x1_{tt}", tag=f"x1_{tt}")
            nc.vector.scalar_tensor_tensor(x1[:, :], s[:, :], v[:, 0:1],
                                           n1w_rep[:, :], op0=OP.mult, op1=OP.mult)
            x1_l.append(x1)
            if dbg and tt == 0:
                nc.sync.dma_start(dbg_t["dbg_x1"][:, :], x1[:, :])
            amax = bpB.tile([128, 1], F32, name=f"am{tt}", tag="am")
            nc.vector.tensor_reduce(amax[:, :], x1[:, :], axis=mybir.AxisListType.X,
                                    op=OP.max, apply_absolute_value=True)
            nc.vector.tensor_scalar(amax[:, :], amax[:, :], 1e-5, None, op0=OP.max)
            sc = bpB.tile([128, 1], F32, name=f"sc{tt}", tag="sc")
            nc.vector.reciprocal(sc[:, :], amax[:, :])
            nc.vector.tensor_scalar(sc[:, :], sc[:, :], 127.0, None, op0=OP.mult)
            scl1 = bpB.tile([128, 1], F32, name=f"scl1_{tt}", tag=f"scl1_{tt}")
            nc.vector.tensor_scalar(scl1[:, :], amax[:, :], g1 / 127.0, None,
                                    op0=OP.mult)
            scl1_l.append(scl1)
            q = scr.tile([128, DM], BF16, name=f"q{tt}", tag="qscr")
            nc.vector.tensor_scalar(q[:, :], x1[:, :], sc[:, 0:1], None, op0=OP.mult)
            nc.vector.tensor_scalar(q[:, :], q[:, :], MAGIC_BF, MAGIC_BF,
                                    op0=OP.add, op1=OP.subtract)
            xq = scr.tile([128, DM], BF16, name=f"xq{tt}", tag="xqscr")
            nc.vector.tensor_scalar(xq[:, :], q[:, :], 127.0, -128.0,
                                    op0=OP.min, op1=OP.max)
            # transpose via PE (8 x [128,128])
            xqT = bpB.tile([128, DM // 128, 128], BF16, name=f"xqT{tt}",
                           tag=f"xqT{tt}")
            for k in range(DM // 128):
                tp = psT.tile([128, 128], BF16, tag="tp")
                nc.tensor.transpose(tp[:, :], xq[:, k * 128:(k + 1) * 128],
                                    ident_bf[:, :])
                nc.scalar.copy(xqT[:, k, :], tp[:, :])
            xqT_l.append(xqT)
    wout_stack.close()

    # ---- FFN mm1 + gelu + quant2 ----
    with tc.tile_pool(name="ffn1", bufs=1) as f1pool:
        for tt in range(NTT):
            f_sb = f1pool.tile([128, DFF], BF16, name=f"f{tt}", tag=f"f{tt}")
            for jc in range(DFF // 512):
                fps = psB.tile([128, 512], F32, tag="mmps")
                for k in range(DM // 128):
                    nc.tensor.matmul(
                        fps[:, :], xqT_l[tt][:, k, :],
                        w1qT_sb[:, k, jc * 512:(jc + 1) * 512],
                        start=(k == 0), stop=(k == DM // 128 - 1))
                nc.scalar.activation(f_sb[:, jc * 512:(jc + 1) * 512], fps[:, :],
                                     AF.Gelu_apprx_tanh, scale=scl1_l[tt][:, 0:1])
            if dbg and tt == 0:
                f32dbg = f1pool.tile([128, DFF], F32, name="f32dbg", tag="f32dbg")
                nc.vector.tensor_copy(f32dbg[:, :], f_sb[:, :])
                nc.sync.dma_start(dbg_t["dbg_f"][:, :], f32dbg[:, :])
            amax2 = bpB.tile([128, 1], F32, name=f"am2{tt}", tag="am2")
            nc.vector.tensor_reduce(amax2[:, :], f_sb[:, :],
                                    axis=mybir.AxisListType.X,
                                    op=OP.max, apply_absolute_value=True)
            nc.vector.tensor_scalar(amax2[:, :], amax2[:, :], 1e-5, None, op0=OP.max)
            sc2 = bpB.tile([128, 1], F32, name=f"sc2{tt}", tag="sc2")
            nc.vector.reciprocal(sc2[:, :], amax2[:, :])
            nc.vector.tensor_scalar(sc2[:, :], sc2[:, :], 127.0, None, op0=OP.mult)
            scl2 = bpB.tile([128, 1], F32, name=f"scl2_{tt}", tag=f"scl2_{tt}")
            nc.vector.tensor_scalar(scl2[:, :], amax2[:, :], g2 / 127.0, None,
                                    op0=OP.mult)
            scl2_l.append(scl2)
            q2 = f1pool.tile([128, DFF], BF16, name=f"q2{tt}", tag="q2scr")
            nc.vector.tensor_scalar(q2[:, :], f_sb[:, :], sc2[:, 0:1], None,
                                    op0=OP.mult)
            nc.vector.tensor_scalar(q2[:, :], q2[:, :], MAGIC_BF, MAGIC_BF,
                                    op0=OP.add, op1=OP.subtract)
            fq = f1pool.tile([128, DFF], BF16, name=f"fq{tt}", tag="fqscr")
            nc.vector.tensor_scalar(fq[:, :], q2[:, :], 127.0, -128.0,
                                    op0=OP.min, op1=OP.max)
            fqT = bpB.tile([128, DFF // 128, 128], BF16, name=f"fqT{tt}",
                           tag=f"fqT{tt}")
            for k in range(DFF // 128):
                tp = psT.tile([128, 128], BF16, tag="tp")
                nc.tensor.transpose(tp[:, :], fq[:, k * 128:(k + 1) * 128],
                                    ident_bf[:, :])
                nc.scalar.copy(fqT[:, k, :], tp[:, :])
            fqT_l.append(fqT)

    # ---- FFN mm2 + residual + rmsnorm2 ----
    with tc.tile_pool(name="ffn2", bufs=1) as f2pool:
        for tt in range(NTT):
            o2 = f2pool.tile([128, DM], F32, name=f"o2{tt}", tag=f"o2{tt}")
            for mc in range(DM // 512):
                ops_ = psB.tile([128, 512], F32, tag="mmps")
                for k in range(DFF // 128):
                    nc.tensor.matmul(
                        ops_[:, :], fqT_l[tt][:, k, :],
                        w2qT_sb[:, k, mc * 512:(mc + 1) * 512],
                        start=(k == 0), stop=(k == DFF // 128 - 1))
                nc.vector.scalar_tensor_tensor(
                    o2[:, mc * 512:(mc + 1) * 512], ops_[:, :], scl2_l[tt][:, 0:1],
                    x1_l[tt][:, mc * 512:(mc + 1) * 512], op0=OP.mult, op1=OP.add)
            sq2 = f2pool.tile([128, DM], F32, name=f"sq2{tt}", tag="sq2scr")
            ssum2 = f2pool.tile([128, 1], F32, name=f"ssum2{tt}", tag="ssum2")
            nc.scalar.activation(sq2[:, :], o2[:, :], AF.Square,
                                 accum_out=ssum2[:, 0:1])
            v2 = f2pool.tile([128, 1], F32, name=f"v2{tt}", tag=f"v2{tt}")
            nc.vector.tensor_scalar(v2[:, :], ssum2[:, :], 1.0 / DM, EPS,
                                    op0=OP.mult, op1=OP.add)
            nc.scalar.activation(v2[:, :], v2[:, :], AF.Sqrt)
            nc.vector.reciprocal(v2[:, :], v2[:, :])
            ot = f2pool.tile([128, DM], F32, name=f"ot{tt}", tag="otscr")
            nc.vector.scalar_tensor_tensor(ot[:, :], o2[:, :], v2[:, 0:1],
                                           n2w_rep[:, :], op0=OP.mult, op1=OP.mult)
            nc.sync.dma_start(out_t[tt * 128:(tt + 1) * 128, :], ot[:, :])


def build_nc(g1, g2, dbg=False):
    from contextlib import ExitStack
    nc = bacc.Bacc("TRN2", target_bir_lowering=False, debug=False,
                   num_devices=NCORES)
    with ExitStack() as ctx:
        tc = ctx.enter_context(tile.TileContext(nc))
        _emit(nc, tc, ctx, g1, g2, dbg)
    nc.compile()
    return nc


def host_prep(inputs):
    bf = ml_dtypes.bfloat16
    f8 = ml_dtypes.float8_e4m3
    x = np.asarray(inputs["x"], np.float32)
    x2d = x.reshape(L, DM)
    w_in = np.asarray(inputs["w_in"], np.float32)
    conv_w = np.asarray(inputs["conv_w"], np.float32)
    conv_b = np.asarray(inputs["conv_b"], np.float32)
    w_xproj = np.asarray(inputs["w_xproj"], np.float32)
    w_dt = np.asarray(inputs["w_dt"], np.float32)
    b_dt = np.asarray(inputs["b_dt"], np.float32)
    A_log = np.asarray(inputs["A_log"], np.float32)
    Dp = np.asarray(inputs["Dp"], np.float32)
    w_out = np.asarray(inputs["w_out"], np.float32)
    n1 = np.asarray(inputs["norm1_w"], np.float32)
    n2 = np.asarray(inputs["norm2_w"], np.float32)
    w1 = np.asarray(inputs["ffn_w1"], np.float32)
    w2 = np.asarray(inputs["ffn_w2"], np.float32)
    b1 = np.asarray(inputs["ffn_b1"], np.float32)
    b2 = np.asarray(inputs["ffn_b2"], np.float32)
    assert np.all(b1 == 0.0) and np.all(b2 == 0.0), "nonzero ffn bias unsupported"

    g1 = float(np.maximum(np.mean(np.abs(w1), dtype=np.float32), 1e-5))
    g2 = float(np.maximum(np.mean(np.abs(w2), dtype=np.float32), 1e-5))
    w1q = np.clip(np.rint(w1 / g1), -1.0, 1.0).astype(np.float32)
    w2q = np.clip(np.rint(w2 / g2), -1.0, 1.0).astype(np.float32)

    xT_bf = np.ascontiguousarray(x2d.T).astype(bf)
    woutT_bf = np.ascontiguousarray(w_out.T).astype(bf)
    w1qT_f8 = np.ascontiguousarray(w1q.T).astype(f8)
    w2qT_f8 = np.ascontiguousarray(w2q.T).astype(f8)
    n1r = np.ascontiguousarray(n1.reshape(1, DM))
    n2r = np.ascontiguousarray(n2.reshape(1, DM))
    A = -np.exp(A_log)

    in_maps = []
    for c in range(NCORES):
        ch = slice(c * DIC, (c + 1) * DIC)
        w_sel = np.concatenate([w_in[c * DIC:(c + 1) * DIC],
                                w_in[DI + c * DIC:DI + (c + 1) * DIC]], axis=0)
        in_maps.append({
            "xT": xT_bf,
            "x_tok": np.ascontiguousarray(x2d[c * LT:(c + 1) * LT]),
            "winT": np.ascontiguousarray(w_sel.T).astype(bf),
            "convw": np.ascontiguousarray(conv_w[ch, 0, :]),
            "convb": np.ascontiguousarray(conv_b[ch].reshape(-1, 1)),
            "wxpT": np.ascontiguousarray(w_xproj[:, ch].T).astype(bf),
            "wdtT": np.ascontiguousarray(w_dt[ch, :].T).astype(bf),
            "bdt": np.ascontiguousarray(b_dt[ch].reshape(-1, 1)),
            "acol": np.ascontiguousarray(A[ch, :]),
            "dpv": np.ascontiguousarray(Dp[ch].reshape(-1, 1)),
            "woutT": woutT_bf,
            "n1w": n1r,
            "n2w": n2r,
            "w1qT": w1qT_f8,
            "w2qT": w2qT_f8,
        })
    return in_maps, g1, g2


def kernel(**inputs) -> np.ndarray:
    in_maps, g1, g2 = host_prep(inputs)
    key = (round(g1, 10), round(g2, 10))
    if key not in _NC_CACHE:
        _NC_CACHE[key] = build_nc(g1, g2)
    nc = _NC_CACHE[key]
    res = run_bass_kernel_spmd(nc, in_maps, core_ids=list(range(NCORES)))
    out = np.concatenate([res.results[c]["out"] for c in range(NCORES)], axis=0)
    return np.ascontiguousarray(out.reshape(1, L, DM).astype(np.float32))


# revision 3
# speedup vs baseline: 52.8877x; 1.0407x over previous
"""Trainium2 Bass kernel v2 for nn_DecoderLayer (Mamba block + BitNet FFN).

Sharding: channel-parallel mamba (256 ch/core) -> AllReduce (xproj rows) ->
DVE tensor_tensor_scan over (d,n) lanes -> AllToAll (d-shard -> t-shard) ->
sequence-parallel out_proj + rmsnorm + BitNet FFN (fp8 ternary weights) ->
each core emits its 256-token slice.

v2: fp8e4 FFN weights (halved HBM/tunnel bytes), w1 prefetched at t0 and
wout/w2 loads overlapped with scan/A2A, bf16 datapath for 2x DVE modes,
B/C replication split across both HWDGE rings, native Softplus, PE
transposes instead of DMA transposes, chunked xT streaming.
"""
import numpy as np
import ml_dtypes

try:
    import jax
    jax.config.update("jax_compilation_cache_dir", "/root/jaxcache")
    jax.config.update("jax_persistent_cache_min_compile_time_secs", 1.0)
except Exception:
    pass

import concourse.bass as bass
import concourse.mybir as mybir
import concourse.tile as tile
from concourse import bacc
from concourse.bass_utils import run_bass_kernel_spmd

BF16 = mybir.dt.bfloat16
F32 = mybir.dt.float32
FP8 = mybir.dt.float8e4
AF = mybir.ActivationFunctionType
OP = mybir.AluOpType

L, DM, DI, DS, DC, DTR, DFF = 2048, 1024, 2048, 16, 4, 64, 4096
EPS = 1e-6
NCORES = 8
DIC = DI // NCORES   # 256 channels per core
NDT = DIC // 128     # 2 d-tiles
LT = L // NCORES     # 256 tokens per core
NTT = LT // 128      # 2 token-tiles
MAGIC = 12582912.0   # 1.5*2^23: x+M-M == rint(x) for |x|<2^22 (f32)
MAGIC_BF = 384.0     # 1.5*2^8: bf16 magic for ints in [-128,127]

_NC_CACHE = {}


def _rep_ap(src):
    """Broadcast a 1-row DRAM slice across 128 partitions."""
    return bass.AP(tensor=src.tensor, offset=src.offset,
                   ap=[[0, 128]] + [list(p) for p in src.ap[1:]])


def _emit(nc, tc, ctx, g1, g2, dbg):
    import contextlib
    RG = [list(range(NCORES))]

    xT = nc.dram_tensor("xT", [DM, L], BF16, kind="ExternalInput")
    x_tok = nc.dram_tensor("x_tok", [LT, DM], F32, kind="ExternalInput")
    winT = nc.dram_tensor("winT", [DM, 2 * 128 * NDT], BF16, kind="ExternalInput")
    convw = nc.dram_tensor("convw", [DIC, DC], F32, kind="ExternalInput")
    convb = nc.dram_tensor("convb", [DIC, 1], F32, kind="ExternalInput")
    wxpT = nc.dram_tensor("wxpT", [DIC, 96], BF16, kind="ExternalInput")
    wdtT = nc.dram_tensor("wdtT", [DTR, DIC], BF16, kind="ExternalInput")
    bdt = nc.dram_tensor("bdt", [DIC, 1], F32, kind="ExternalInput")
    acol = nc.dram_tensor("acol", [DIC, DS], F32, kind="ExternalInput")
    dpv = nc.dram_tensor("dpv", [DIC, 1], F32, kind="ExternalInput")
    woutT = nc.dram_tensor("woutT", [DI, DM], BF16, kind="ExternalInput")
    n1w = nc.dram_tensor("n1w", [1, DM], F32, kind="ExternalInput")
    n2w = nc.dram_tensor("n2w", [1, DM], F32, kind="ExternalInput")
    w1qT = nc.dram_tensor("w1qT", [DM, DFF], FP8, kind="ExternalInput")
    w2qT = nc.dram_tensor("w2qT", [DFF, DM], FP8, kind="ExternalInput")
    out_t = nc.dram_tensor("out", [LT, DM], F32, kind="ExternalOutput")
    dbg_t = {}
    if dbg:
        dbg_t["dbg_u"] = nc.dram_tensor("dbg_u", [128, L], F32, kind="ExternalOutput")
        dbg_t["dbg_delta"] = nc.dram_tensor("dbg_delta", [128, L], F32, kind="ExternalOutput")
        dbg_t["dbg_dbl"] = nc.dram_tensor("dbg_dbl", [96, L], F32, kind="ExternalOutput")
        dbg_t["dbg_yhat"] = nc.dram_tensor("dbg_yhat", [128, L], F32, kind="ExternalOutput")
        dbg_t["dbg_x1"] = nc.dram_tensor("dbg_x1", [128, DM], F32, kind="ExternalOutput")
        dbg_t["dbg_f"] = nc.dram_tensor("dbg_f", [128, DFF], F32, kind="ExternalOutput")

    singles = ctx.enter_context(tc.tile_pool(name="singles", bufs=1))
    dram = ctx.enter_context(tc.tile_pool(name="dram", bufs=1, space="DRAM"))
    psA_stack = contextlib.ExitStack()
    psum_small = psA_stack.enter_context(
        tc.tile_pool(name="psA", bufs=3, space="PSUM"))
    # w1-only prefetch pool (w2 gets its own late pool)
    ffnwpool = ctx.enter_context(tc.tile_pool(name="ffnwp", bufs=1))
    bpool = ctx.enter_context(tc.tile_pool(name="bpool", bufs=1))
    act_stack = contextlib.ExitStack()
    actpool = act_stack.enter_context(tc.tile_pool(name="acts", bufs=1))
    xp_stack = contextlib.ExitStack()
    xppool = xp_stack.enter_context(tc.tile_pool(name="xpp", bufs=1))
    conv_stack = contextlib.ExitStack()
    convpool = conv_stack.enter_context(tc.tile_pool(name="convp", bufs=2))
    init_stack = contextlib.ExitStack()
    init_pool = init_stack.enter_context(tc.tile_pool(name="init", bufs=2))

    # ---- winT (Act ring) + xT chunks (SP ring) first: in_proj starts asap
    winT_sb = init_pool.tile([128, 8, 2 * 128 * NDT], BF16, name="winT")
    nc.scalar.dma_start(winT_sb[:, :, :],
                        winT.rearrange("(k p) m -> p k m", p=128))
    xTr0 = xT.rearrange("(k p) l -> p k l", p=128)
    xT_cs = []
    for c in range(L // 512):
        xT_c = init_pool.tile([128, 8, 512], BF16, name=f"xTc{c}")
        nc.sync.dma_start(xT_c[:, :, :], xTr0[:, :, c * 512:(c + 1) * 512])
        xT_cs.append(xT_c)

    # ---- small per-partition constants (SP ring, after winT) ----
    convw_sb, convb_sb, bdt_sb, acol_sb, dp_sb = [], [], [], [], []
    for dt in range(NDT):
        sl = slice(dt * 128, (dt + 1) * 128)
        t1 = singles.tile([128, DC], F32, name=f"cw{dt}")
        nc.sync.dma_start(t1[:, :], convw[sl, :])
        convw_sb.append(t1)
        t2 = singles.tile([128, 1], F32, name=f"cb{dt}")
        nc.sync.dma_start(t2[:, :], convb[sl, :])
        convb_sb.append(t2)
        t3 = singles.tile([128, 1], F32, name=f"bd{dt}")
        nc.sync.dma_start(t3[:, :], bdt[sl, :])
        bdt_sb.append(t3)
        t4 = singles.tile([128, DS], F32, name=f"ac{dt}")
        nc.sync.dma_start(t4[:, :], acol[sl, :])
        acol_sb.append(t4)
        t5 = singles.tile([128, 1], F32, name=f"dp{dt}")
        nc.sync.dma_start(t5[:, :], dpv[sl, :])
        dp_sb.append(t5)
    wxpT_sb = singles.tile([128, NDT, 96], BF16)
    nc.sync.dma_start(wxpT_sb[:, :, :], wxpT.rearrange("(k p) m -> p k m", p=128))
    wdtT_sb = singles.tile([DTR, DIC], BF16)
    nc.sync.dma_start(wdtT_sb[:, :], wdtT[:, :])
    ident_bf = singles.tile([128, 128], BF16)
    from concourse.masks import make_identity
    make_identity(nc, ident_bf[:, :])

    # ============ PHASE A: in_proj + conv + xproj, chunk-pipelined ============
    xTr = xT.rearrange("(k p) l -> p k l", p=128)
    ar_i = dram.tile([96, L], BF16)
    u_pad_c, u_act_c, zs, dbl_c = [], [], [], []
    for dt in range(NDT):
        u_pad_c.append([convpool.tile([128, 515], BF16, name=f"up{dt}_{c}")
                        for c in range(L // 512)])
        u_act_c.append([actpool.tile([128, 512], BF16, name=f"ua{dt}_{c}")
                        for c in range(L // 512)])
        zs.append(actpool.tile([128, L], BF16, name=f"zs{dt}"))
        nc.vector.memset(u_pad_c[dt][0][:, 0:3], 0.0)
    for c in range(L // 512):
        dbl_c.append(xppool.tile([96, 512], BF16, name=f"dbl{c}"))

    # pass 1: all u-tile matmuls (PE queue flows without xproj stalls)
    for c in range(L // 512):
        xT_c = xT_cs[c]
        for mt in range(NDT):
            ps = psum_small.tile([128, 512], F32, tag="psA")
            for k in range(8):
                nc.tensor.matmul(
                    ps[:, :],
                    winT_sb[:, k, mt * 128:(mt + 1) * 128],
                    xT_c[:, k, :],
                    start=(k == 0), stop=(k == 7))
            nc.scalar.copy(u_pad_c[mt][c][:, 3:515], ps[:, :])
            if c + 1 < L // 512:
                # halo: first 3 cols of next chunk = last 3 of this chunk
                nc.vector.tensor_copy(u_pad_c[mt][c + 1][:, 0:3],
                                      u_pad_c[mt][c][:, 512:515])
    # pass 2: conv chunks (each self-contained thanks to the halo)
    for c in range(L // 512):
        for dt in range(NDT):
            up = u_pad_c[dt][c]
            ca = convpool.tile([128, 512], F32, tag="cva")
            cb = convpool.tile([128, 512], F32, tag="cvb")
            nc.vector.tensor_scalar_mul(ca[:, :], up[:, 0:512],
                                        convw_sb[dt][:, 0:1])
            nc.vector.scalar_tensor_tensor(
                cb[:, :], up[:, 1:513], convw_sb[dt][:, 1:2],
                ca[:, :], op0=OP.mult, op1=OP.add)
            nc.vector.scalar_tensor_tensor(
                ca[:, :], up[:, 2:514], convw_sb[dt][:, 2:3],
                cb[:, :], op0=OP.mult, op1=OP.add)
            nc.vector.scalar_tensor_tensor(
                cb[:, :], up[:, 3:515], convw_sb[dt][:, 3:4],
                ca[:, :], op0=OP.mult, op1=OP.add)
            nc.scalar.activation(u_act_c[dt][c][:, :], cb[:, :], AF.Silu,
                                 bias=convb_sb[dt][:, 0:1])
    # pass 3: xproj partials + AR staging
    for c in range(L // 512):
        lo, hi = c * 512, (c + 1) * 512
        ps96 = psum_small.tile([96, 512], F32, tag="ps96")
        for kt in range(NDT):
            nc.tensor.matmul(
                ps96[:, :],
                wxpT_sb[:, kt, :],
                u_act_c[kt][c][:, :],
                start=(kt == 0), stop=(kt == NDT - 1))
        nc.scalar.copy(dbl_c[c][:, :], ps96[:, :])
        nc.sync.dma_start(ar_i[:, lo:hi], dbl_c[c][:, :])
    # z m-tiles: computed during the AllReduce window
    for c in range(L // 512):
        lo, hi = c * 512, (c + 1) * 512
        for mt in range(NDT, 2 * NDT):
            ps = psum_small.tile([128, 512], F32, tag="psA")
            for k in range(8):
                nc.tensor.matmul(
                    ps[:, :],
                    winT_sb[:, k, mt * 128:(mt + 1) * 128],
                    xT_cs[c][:, k, :],
                    start=(k == 0), stop=(k == 7))
            nc.scalar.activation(
                zs[mt - NDT][:, lo:hi], ps[:, :], AF.Silu)
    # ---- phase-B prefetches: after Phase A ring traffic
    w1qT_sb = ffnwpool.tile([128, DM // 128, DFF], FP8)
    nc.scalar.dma_start(w1qT_sb[:, :, :], w1qT.rearrange("(k p) j -> p k j", p=128))
    x_tok_sb = bpool.tile([128, NTT, DM], F32)
    nc.scalar.dma_start(x_tok_sb[:, :, :], x_tok.rearrange("(tt p) m -> p tt m", p=128))
    n1w_rep = bpool.tile([128, DM], F32)
    nc.scalar.dma_start(n1w_rep[:, :], _rep_ap(n1w[0:1, :]))
    n2w_rep = bpool.tile([128, DM], F32)
    nc.scalar.dma_start(n2w_rep[:, :], _rep_ap(n2w[0:1, :]))
    init_stack.close()
    conv_stack.close()
    if dbg:
        u32 = actpool.tile([128, L], F32, name="u32dbg")
        for c in range(L // 512):
            nc.vector.tensor_copy(u32[:, c * 512:(c + 1) * 512],
                                  u_act_c[0][c][:, :])
        nc.sync.dma_start(dbg_t["dbg_u"][:, :], u32[:, :])

    # ================= AllReduce (bf16) =================
    ar_o = dram.tile([96, L], BF16, addr_space="Shared")
    nc.gpsimd.collective_compute("AllReduce", OP.add, replica_groups=RG,
                                 ins=[ar_i.opt()], outs=[ar_o.opt()])
    # dt rows for the delta matmul; B/C rows are read straight from ar_o
    dt_bf = xppool.tile([DTR, L], BF16)
    nc.sync.dma_start(dt_bf[:, :], ar_o[0:DTR, :])
    if dbg:
        dbl_sb = xppool.tile([96, L], F32)
        nc.sync.dma_start(dbl_sb[:, :], ar_o[:, :])
        nc.sync.dma_start(dbg_t["dbg_dbl"][:, :], dbl_sb[:, :])
    bcb = ar_o[64:96, :]

    # ================= delta = softplus(wdt @ dt + bdt), bf16 ============
    delta = []
    for dt in range(NDT):
        dl = actpool.tile([128, L], BF16, name=f"delta{dt}")
        for c in range(L // 512):
            ps = psum_small.tile([128, 512], F32, tag="psA")
            nc.tensor.matmul(
                ps[:, :],
                wdtT_sb[:, dt * 128:(dt + 1) * 128],
                dt_bf[:, c * 512:(c + 1) * 512],
                start=True, stop=True)
            nc.scalar.activation(dl[:, c * 512:(c + 1) * 512], ps[:, :],
                                 AF.Exp, bias=bdt_sb[dt][:, 0:1])
        nc.scalar.activation(dl[:, :], dl[:, :], AF.Ln, bias=1.0)
        delta.append(dl)
    if dbg:
        d32 = actpool.tile([128, L], F32, name="d32dbg")
        nc.vector.tensor_copy(d32[:, :], delta[0][:, :])
        nc.sync.dma_start(dbg_t["dbg_delta"][:, :], d32[:, :])

    xp_stack.close()
    # delta*u in bf16 for the scan input product (both bf16 -> 2x DVE)
    du_bf = []
    for dt in range(NDT):
        db = actpool.tile([128, L], BF16, name=f"dubf{dt}")
        for c in range(L // 512):
            nc.vector.tensor_tensor(db[:, c * 512:(c + 1) * 512],
                                    delta[dt][:, c * 512:(c + 1) * 512],
                                    u_act_c[dt][c][:, :], op=OP.mult)
        du_bf.append(db)

    # ================= scan over n (16 states) =================
    psA_stack.close()
    yps_stack = contextlib.ExitStack()
    y_ps_pool = yps_stack.enter_context(
        tc.tile_pool(name="yps", bufs=1, space="PSUM"))
    y_ps = [y_ps_pool.tile([128, L], F32, name=f"yps{dt}") for dt in range(NDT)]

    scanp = act_stack.enter_context(tc.tile_pool(name="scanp", bufs=3))
    repp = act_stack.enter_context(tc.tile_pool(name="repp", bufs=3))
    a2a_i = [dram.tile([DI // 2, LT], BF16, name=f"a2ai{d}") for d in range(NDT)]
    a2a_o = [dram.tile([DI // 2, LT], BF16, name=f"a2ao{d}") for d in range(NDT)]
    for dt in range(NDT):
        for n in range(DS):
            brep = repp.tile([128, L], BF16, name=f"brep{dt}_{n}", tag="brep")
            nc.sync.dma_start(brep[:, :], _rep_ap(bcb[n:n + 1, :]))
            crep = repp.tile([128, L], BF16, name=f"crep{dt}_{n}", tag="crep")
            nc.scalar.dma_start(crep[:, :], _rep_ap(bcb[16 + n:17 + n, :]))
            dA = scanp.tile([128, L], BF16, name=f"dA{n}_{dt}", tag="dA")
            nc.scalar.activation(dA[:, :], delta[dt][:, :], AF.Exp,
                                 scale=acol_sb[dt][:, n:n + 1])
            dBu = scanp.tile([128, L], BF16, name=f"dBu{n}_{dt}", tag="dBu")
            nc.vector.tensor_tensor(dBu[:, :], du_bf[dt][:, :], brep[:, :],
                                    op=OP.mult)
            h = scanp.tile([128, L], BF16, name=f"h{n}_{dt}", tag="h")
            nc.vector.tensor_tensor_scan(h[:, :], dA[:, :], dBu[:, :], 0.0,
                                         OP.mult, OP.add)
            yt = scanp.tile([128, L], BF16, name=f"yt{n}_{dt}", tag="yt")
            eng_tt = nc.gpsimd if n % 2 == 1 else nc.vector
            eng_tt.tensor_tensor(yt[:, :], h[:, :], crep[:, :], op=OP.mult)
            for c in range(L // 512):
                nc.tensor.matmul(
                    y_ps[dt][:, c * 512:(c + 1) * 512],
                    ident_bf[:, :],
                    yt[:, c * 512:(c + 1) * 512],
                    start=(n == 0), stop=(n == DS - 1),
                    skip_group_check=True)
        # gate for this dt, then its half-A2A (overlaps the other dt's scan)
        t1 = scanp.tile([128, L], BF16, name=f"yg{dt}", tag="dA")
        for c in range(L // 512):
            nc.vector.scalar_tensor_tensor(
                t1[:, c * 512:(c + 1) * 512], u_act_c[dt][c][:, :],
                dp_sb[dt][:, 0:1], y_ps[dt][:, c * 512:(c + 1) * 512],
                op0=OP.mult, op1=OP.add)
        yh = scanp.tile([128, L], BF16, name=f"yhat{dt}", tag="dBu")
        nc.vector.tensor_tensor(yh[:, :], t1[:, :], zs[dt][:, :], op=OP.mult)
        if dbg and dt == 0:
            yh32 = scanp.tile([128, L], F32, name="yh32", tag="yh32")
            nc.vector.tensor_copy(yh32[:, :], yh[:, :])
            nc.sync.dma_start(dbg_t["dbg_yhat"][:, :], yh32[:, :])
        # scatter my 128-ch rows into (8 token-blocks x 128) layout
        nc.sync.dma_start(
            a2a_i[dt].rearrange("(j c) t -> c j t", c=128)[:, :, :],
            yh.rearrange("c (j t) -> c j t", j=NCORES))
        nc.gpsimd.collective_compute("AllToAll", OP.bypass, replica_groups=RG,
                                     ins=[a2a_i[dt].opt()],
                                     outs=[a2a_o[dt].opt()])

    # ================= PHASE B (sequence-parallel, my LT tokens) ==========
    yps_stack.close()
    act_stack.close()
    psB = ctx.enter_context(tc.tile_pool(name="psB", bufs=2, space="PSUM"))
    psT = ctx.enter_context(tc.tile_pool(name="psT", bufs=2, space="PSUM"))
    scr = ctx.enter_context(tc.tile_pool(name="scr", bufs=1))
    bpB = ctx.enter_context(tc.tile_pool(name="bpB", bufs=1))
    w2pool = ctx.enter_context(tc.tile_pool(name="w2p", bufs=1))
    # wout + w2 loads: start as soon as scan-era SBUF frees; overlap A2A wait
    wout_stack = contextlib.ExitStack()
    woutpool = wout_stack.enter_context(tc.tile_pool(name="woutp", bufs=1))
    woutT_sb = woutpool.tile([128, DI // 128, DM], BF16)
    nc.sync.dma_start(woutT_sb[:, :, :], woutT.rearrange("(k p) m -> p k m", p=128))
    w2qT_sb = w2pool.tile([128, DFF // 128, DM], FP8)
    nc.scalar.dma_start(w2qT_sb[:, :, :], w2qT.rearrange("(k p) m -> p k m", p=128))

    x1_l, scl1_l, xqT_l, fqT_l, scl2_l = [], [], [], [], []

    # ---- out_proj + rmsnorm1 + quant1 ----
    with tc.tile_pool(name="oproj", bufs=1) as opool:
        yfull = opool.tile([128, NDT, NCORES, LT], BF16)
        for d in range(NDT):
            nc.sync.dma_start(yfull[:, d, :, :],
                              a2a_o[d].rearrange("(j p) t -> p j t", p=128))
        for tt in range(NTT):
            hps = psB.tile([128, DM], F32, tag="hps")
            for c in range(DM // 512):
                kk = 0
                for d in range(NDT):
                    for j in range(NCORES):
                        nc.tensor.matmul(
                            hps[:, c * 512:(c + 1) * 512],
                            yfull[:, d, j, tt * 128:(tt + 1) * 128],
                            woutT_sb[:, j * NDT + d, c * 512:(c + 1) * 512],
                            start=(kk == 0), stop=(kk == DI // 128 - 1))
                        kk += 1
            s = scr.tile([128, DM], F32, name=f"s{tt}", tag="sscr")
            nc.vector.tensor_tensor(s[:, :], x_tok_sb[:, tt, :], hps[:, :], op=OP.add)
            sq = scr.tile([128, DM], F32, name=f"sq{tt}", tag="sqscr")
            ssum = bpB.tile([128, 1], F32, name=f"ssum{tt}", tag="ssum")
            nc.scalar.activation(sq[:, :], s[:, :], AF.Square, accum_out=ssum[:, 0:1])
            v = bpB.tile([128, 1], F32, name=f"v{tt}", tag=f"v{tt}")
            nc.vector.tensor_scalar(v[:, :], ssum[:, :], 1.0 / DM, EPS,
                                    op0=OP.mult, op1=OP.add)
            nc.scalar.activation(v[:, :], v[:, :], AF.Sqrt)
            nc.vector.reciprocal(v[:, :], v[:, :])
            x1 = bpB.tile([128, DM], F32, name=f"x1_{tt}", tag=f"x1_{tt}")
            nc.vector.scalar_tensor_tensor(x1[:, :], s[:, :], v[:, 0:1],
                                           n1w_rep[:, :], op0=OP.mult, op1=OP.mult)
            x1_l.append(x1)
            if dbg and tt == 0:
                nc.sync.dma_start(dbg_t["dbg_x1"][:, :], x1[:, :])
            amax = bpB.tile([128, 1], F32, name=f"am{tt}", tag="am")
            nc.vector.tensor_reduce(amax[:, :], x1[:, :], axis=mybir.AxisListType.X,
                                    op=OP.max, apply_absolute_value=True)
            nc.vector.tensor_scalar(amax[:, :], amax[:, :], 1e-5, None, op0=OP.max)
            sc = bpB.tile([128, 1], F32, name=f"sc{tt}", tag="sc")
            nc.vector.reciprocal(sc[:, :], amax[:, :])
            nc.vector.tensor_scalar(sc[:, :], sc[:, :], 127.0, None, op0=OP.mult)
            scl1 = bpB.tile([128, 1], F32, name=f"scl1_{tt}", tag=f"scl1_{tt}")
            nc.vector.tensor_scalar(scl1[:, :], amax[:, :], g1 / 127.0, None,
                                    op0=OP.mult)
            scl1_l.append(scl1)
            q = scr.tile([128, DM], BF16, name=f"q{tt}", tag="qscr")
            nc.vector.tensor_scalar(q[:, :], x1[:, :], sc[:, 0:1], None, op0=OP.mult)
            nc.vector.tensor_scalar(q[:, :], q[:, :], MAGIC_BF, MAGIC_BF,
                                    op0=OP.add, op1=OP.subtract)
            xq = scr.tile([128, DM], BF16, name=f"xq{tt}", tag="xqscr")
            nc.vector.tensor_scalar(xq[:, :], q[:, :], 127.0, -128.0,
                                    op0=OP.min, op1=OP.max)
            # transpose via PE (8 x [128,128])
            xqT = bpB.tile([128, DM // 128, 128], BF16, name=f"xqT{tt}",
                           tag=f"xqT{tt}")
            for k in range(DM // 128):
                tp = psT.tile([128, 128], BF16, tag="tp")
                nc.tensor.transpose(tp[:, :], xq[:, k * 128:(k + 1) * 128],
                                    ident_bf[:, :])
                nc.scalar.copy(xqT[:, k, :], tp[:, :])
            xqT_l.append(xqT)
    wout_stack.close()

    # ---- FFN mm1 + gelu + quant2 ----
    with tc.tile_pool(name="ffn1", bufs=1) as f1pool:
        for tt in range(NTT):
            f_sb = f1pool.tile([128, DFF], BF16, name=f"f{tt}", tag=f"f{tt}")
            for jc in range(DFF // 512):
                fps = psB.tile([128, 512], F32, tag="mmps")
                for k in range(DM // 128):
                    nc.tensor.matmul(
                        fps[:, :], xqT_l[tt][:, k, :],
                        w1qT_sb[:, k, jc * 512:(jc + 1) * 512],
                        start=(k == 0), stop=(k == DM // 128 - 1))
                nc.scalar.activation(f_sb[:, jc * 512:(jc + 1) * 512], fps[:, :],
                                     AF.Gelu_apprx_tanh, scale=scl1_l[tt][:, 0:1])
            if dbg and tt == 0:
                f32dbg = f1pool.tile([128, DFF], F32, name="f32dbg", tag="f32dbg")
                nc.vector.tensor_copy(f32dbg[:, :], f_sb[:, :])
                nc.sync.dma_start(dbg_t["dbg_f"][:, :], f32dbg[:, :])
            amax2 = bpB.tile([128, 1], F32, name=f"am2{tt}", tag="am2")
            nc.vector.tensor_reduce(amax2[:, :], f_sb[:, :],
                                    axis=mybir.AxisListType.X,
                                    op=OP.max, apply_absolute_value=True)
            nc.vector.tensor_scalar(amax2[:, :], amax2[:, :], 1e-5, None, op0=OP.max)
            sc2 = bpB.tile([128, 1], F32, name=f"sc2{tt}", tag="sc2")
            nc.vector.reciprocal(sc2[:, :], amax2[:, :])
            nc.vector.tensor_scalar(sc2[:, :], sc2[:, :], 127.0, None, op0=OP.mult)
            scl2 = bpB.tile([128, 1], F32, name=f"scl2_{tt}", tag=f"scl2_{tt}")
            nc.vector.tensor_scalar(scl2[:, :], amax2[:, :], g2 / 127.0, None,
                                    op0=OP.mult)
            scl2_l.append(scl2)
            q2 = f1pool.tile([128, DFF], BF16, name=f"q2{tt}", tag="q2scr")
            nc.vector.tensor_scalar(q2[:, :], f_sb[:, :], sc2[:, 0:1], None,
                                    op0=OP.mult)
            nc.vector.tensor_scalar(q2[:, :], q2[:, :], MAGIC_BF, MAGIC_BF,
                                    op0=OP.add, op1=OP.subtract)
            fq = f1pool.tile([128, DFF], BF16, name=f"fq{tt}", tag="fqscr")
            nc.vector.tensor_scalar(fq[:, :], q2[:, :], 127.0, -128.0,
                                    op0=OP.min, op1=OP.max)
            fqT = bpB.tile([128, DFF // 128, 128], BF16, name=f"fqT{tt}",
                           tag=f"fqT{tt}")
            for k in range(DFF // 128):
                tp = psT.tile([128, 128], BF16, tag="tp")
                nc.tensor.transpose(tp[:, :], fq[:, k * 128:(k + 1) * 128],
                                    ident_bf[:, :])
                nc.scalar.copy(fqT[:, k, :], tp[:, :])
            fqT_l.append(fqT)

    # ---- FFN mm2 + residual + rmsnorm2 ----
    with tc.tile_pool(name="ffn2", bufs=1) as f2pool:
        for tt in range(NTT):
            o2 = f2pool.tile([128, DM], F32, name=f"o2{tt}", tag=f"o2{tt}")
            for mc in range(DM // 512):
                ops_ = psB.tile([128, 512], F32, tag="mmps")
                for k in range(DFF // 128):
                    nc.tensor.matmul(
                        ops_[:, :], fqT_l[tt][:, k, :],
                        w2qT_sb[:, k, mc * 512:(mc + 1) * 512],
                        start=(k == 0), stop=(k == DFF // 128 - 1))
                nc.vector.scalar_tensor_tensor(
                    o2[:, mc * 512:(mc + 1) * 512], ops_[:, :], scl2_l[tt][:, 0:1],
                    x1_l[tt][:, mc * 512:(mc + 1) * 512], op0=OP.mult, op1=OP.add)
            sq2 = f2pool.tile([128, DM], F32, name=f"sq2{tt}", tag="sq2scr")
            ssum2 = f2pool.tile([128, 1], F32, name=f"ssum2{tt}", tag="ssum2")
            nc.scalar.activation(sq2[:, :], o2[:, :], AF.Square,
                                 accum_out=ssum2[:, 0:1])
            v2 = f2pool.tile([128, 1], F32, name=f"v2{tt}", tag=f"v2{tt}")
            nc.vector.tensor_scalar(v2[:, :], ssum2[:, :], 1.0 / DM, EPS,
                                    op0=OP.mult, op1=OP.add)
            nc.scalar.activation(v2[:, :], v2[:, :], AF.Sqrt)
            nc.vector.reciprocal(v2[:, :], v2[:, :])
            ot = f2pool.tile([128, DM], F32, name=f"ot{tt}", tag="otscr")
            nc.vector.scalar_tensor_tensor(ot[:, :], o2[:, :], v2[:, 0:1],
                                           n2w_rep[:, :], op0=OP.mult, op1=OP.mult)
            nc.sync.dma_start(out_t[tt * 128:(tt + 1) * 128, :], ot[:, :])


def build_nc(g1, g2, dbg=False):
    from contextlib import ExitStack
    nc = bacc.Bacc("TRN2", target_bir_lowering=False, debug=False,
                   num_devices=NCORES)
    with ExitStack() as ctx:
        tc = ctx.enter_context(tile.TileContext(nc))
        _emit(nc, tc, ctx, g1, g2, dbg)
    nc.compile()
    return nc


def host_prep(inputs):
    bf = ml_dtypes.bfloat16
    f8 = ml_dtypes.float8_e4m3
    x = np.asarray(inputs["x"], np.float32)
    x2d = x.reshape(L, DM)
    w_in = np.asarray(inputs["w_in"], np.float32)
    conv_w = np.asarray(inputs["conv_w"], np.float32)
    conv_b = np.asarray(inputs["conv_b"], np.float32)
    w_xproj = np.asarray(inputs["w_xproj"], np.float32)
    w_dt = np.asarray(inputs["w_dt"], np.float32)
    b_dt = np.asarray(inputs["b_dt"], np.float32)
    A_log = np.asarray(inputs["A_log"], np.float32)
    Dp = np.asarray(inputs["Dp"], np.float32)
    w_out = np.asarray(inputs["w_out"], np.float32)
    n1 = np.asarray(inputs["norm1_w"], np.float32)
    n2 = np.asarray(inputs["norm2_w"], np.float32)
    w1 = np.asarray(inputs["ffn_w1"], np.float32)
    w2 = np.asarray(inputs["ffn_w2"], np.float32)
    b1 = np.asarray(inputs["ffn_b1"], np.float32)
    b2 = np.asarray(inputs["ffn_b2"], np.float32)
    assert np.all(b1 == 0.0) and np.all(b2 == 0.0), "nonzero ffn bias unsupported"

    g1 = float(np.maximum(np.mean(np.abs(w1), dtype=np.float32), 1e-5))
    g2 = float(np.maximum(np.mean(np.abs(w2), dtype=np.float32), 1e-5))
    w1q = np.clip(np.rint(w1 / g1), -1.0, 1.0).astype(np.float32)
    w2q = np.clip(np.rint(w2 / g2), -1.0, 1.0).astype(np.float32)

    xT_bf = np.ascontiguousarray(x2d.T).astype(bf)
    woutT_bf = np.ascontiguousarray(w_out.T).astype(bf)
    w1qT_f8 = np.ascontiguousarray(w1q.T).astype(f8)
    w2qT_f8 = np.ascontiguousarray(w2q.T).astype(f8)
    n1r = np.ascontiguousarray(n1.reshape(1, DM))
    n2r = np.ascontiguousarray(n2.reshape(1, DM))
    A = -np.exp(A_log)

    in_maps = []
    for c in range(NCORES):
        ch = slice(c * DIC, (c + 1) * DIC)
        w_sel = np.concatenate([w_in[c * DIC:(c + 1) * DIC],
                                w_in[DI + c * DIC:DI + (c + 1) * DIC]], axis=0)
        in_maps.append({
            "xT": xT_bf,
            "x_tok": np.ascontiguousarray(x2d[c * LT:(c + 1) * LT]),
            "winT": np.ascontiguousarray(w_sel.T).astype(bf),
            "convw": np.ascontiguousarray(conv_w[ch, 0, :]),
            "convb": np.ascontiguousarray(conv_b[ch].reshape(-1, 1)),
            "wxpT": np.ascontiguousarray(w_xproj[:, ch].T).astype(bf),
            "wdtT": np.ascontiguousarray(w_dt[ch, :].T).astype(bf),
            "bdt": np.ascontiguousarray(b_dt[ch].reshape(-1, 1)),
            "acol": np.ascontiguousarray(A[ch, :]),
            "dpv": np.ascontiguousarray(Dp[ch].reshape(-1, 1)),
            "woutT": woutT_bf,
            "n1w": n1r,
            "n2w": n2r,
            "w1qT": w1qT_f8,
            "w2qT": w2qT_f8,
        })
    return in_maps, g1, g2


def kernel(**inputs) -> np.ndarray:
    in_maps, g1, g2 = host_prep(inputs)
    key = (round(g1, 10), round(g2, 10))
    if key not in _NC_CACHE:
        _NC_CACHE[key] = build_nc(g1, g2)
    nc = _NC_CACHE[key]
    res = run_bass_kernel_spmd(nc, in_maps, core_ids=list(range(NCORES)))
    out = np.concatenate([res.results[c]["out"] for c in range(NCORES)], axis=0)
    return np.ascontiguousarray(out.reshape(1, L, DM).astype(np.float32))


# revision 4
# speedup vs baseline: 53.6013x; 1.0135x over previous
"""Trainium2 Bass kernel v2 for nn_DecoderLayer (Mamba block + BitNet FFN).

Sharding: channel-parallel mamba (256 ch/core) -> AllReduce (xproj rows) ->
DVE tensor_tensor_scan over (d,n) lanes -> AllToAll (d-shard -> t-shard) ->
sequence-parallel out_proj + rmsnorm + BitNet FFN (fp8 ternary weights) ->
each core emits its 256-token slice.

v2: fp8e4 FFN weights (halved HBM/tunnel bytes), w1 prefetched at t0 and
wout/w2 loads overlapped with scan/A2A, bf16 datapath for 2x DVE modes,
B/C replication split across both HWDGE rings, native Softplus, PE
transposes instead of DMA transposes, chunked xT streaming.
"""
import numpy as np
import ml_dtypes

try:
    import jax
    jax.config.update("jax_compilation_cache_dir", "/root/jaxcache")
    jax.config.update("jax_persistent_cache_min_compile_time_secs", 1.0)
except Exception:
    pass

import concourse.bass as bass
import concourse.mybir as mybir
import concourse.tile as tile
from concourse import bacc
from concourse.bass_utils import run_bass_kernel_spmd

BF16 = mybir.dt.bfloat16
F32 = mybir.dt.float32
FP8 = mybir.dt.float8e4
AF = mybir.ActivationFunctionType
OP = mybir.AluOpType

L, DM, DI, DS, DC, DTR, DFF = 2048, 1024, 2048, 16, 4, 64, 4096
EPS = 1e-6
NCORES = 8
DIC = DI // NCORES   # 256 channels per core
NDT = DIC // 128     # 2 d-tiles
LT = L // NCORES     # 256 tokens per core
NTT = LT // 128      # 2 token-tiles
MAGIC = 12582912.0   # 1.5*2^23: x+M-M == rint(x) for |x|<2^22 (f32)
MAGIC_BF = 384.0     # 1.5*2^8: bf16 magic for ints in [-128,127]

_NC_CACHE = {}


def _rep_ap(src):
    """Broadcast a 1-row DRAM slice across 128 partitions."""
    return bass.AP(tensor=src.tensor, offset=src.offset,
                   ap=[[0, 128]] + [list(p) for p in src.ap[1:]])


def _emit(nc, tc, ctx, g1, g2, dbg):
    import contextlib
    RG = [list(range(NCORES))]

    xT = nc.dram_tensor("xT", [DM, L], BF16, kind="ExternalInput")
    x_tok = nc.dram_tensor("x_tok", [LT, DM], F32, kind="ExternalInput")
    winT = nc.dram_tensor("winT", [DM, 2 * 128 * NDT], BF16, kind="ExternalInput")
    convw = nc.dram_tensor("convw", [DIC, DC], F32, kind="ExternalInput")
    convb = nc.dram_tensor("convb", [DIC, 1], F32, kind="ExternalInput")
    wxpT = nc.dram_tensor("wxpT", [DIC, 96], BF16, kind="ExternalInput")
    wdtT = nc.dram_tensor("wdtT", [DTR, DIC], BF16, kind="ExternalInput")
    bdt = nc.dram_tensor("bdt", [DIC, 1], F32, kind="ExternalInput")
    acol = nc.dram_tensor("acol", [DIC, DS], F32, kind="ExternalInput")
    dpv = nc.dram_tensor("dpv", [DIC, 1], F32, kind="ExternalInput")
    woutT = nc.dram_tensor("woutT", [DI, DM], BF16, kind="ExternalInput")
    n1w = nc.dram_tensor("n1w", [1, DM], F32, kind="ExternalInput")
    n2w = nc.dram_tensor("n2w", [1, DM], F32, kind="ExternalInput")
    w1qT = nc.dram_tensor("w1qT", [DM, DFF], FP8, kind="ExternalInput")
    w2qT = nc.dram_tensor("w2qT", [DFF, DM], FP8, kind="ExternalInput")
    out_t = nc.dram_tensor("out", [LT, DM], F32, kind="ExternalOutput")
    dbg_t = {}
    if dbg:
        dbg_t["dbg_u"] = nc.dram_tensor("dbg_u", [128, L], F32, kind="ExternalOutput")
        dbg_t["dbg_delta"] = nc.dram_tensor("dbg_delta", [128, L], F32, kind="ExternalOutput")
        dbg_t["dbg_dbl"] = nc.dram_tensor("dbg_dbl", [96, L], F32, kind="ExternalOutput")
        dbg_t["dbg_yhat"] = nc.dram_tensor("dbg_yhat", [128, L], F32, kind="ExternalOutput")
        dbg_t["dbg_x1"] = nc.dram_tensor("dbg_x1", [128, DM], F32, kind="ExternalOutput")
        dbg_t["dbg_f"] = nc.dram_tensor("dbg_f", [128, DFF], F32, kind="ExternalOutput")

    singles = ctx.enter_context(tc.tile_pool(name="singles", bufs=1))
    dram = ctx.enter_context(tc.tile_pool(name="dram", bufs=1, space="DRAM"))
    psA_stack = contextlib.ExitStack()
    psum_small = psA_stack.enter_context(
        tc.tile_pool(name="psA", bufs=3, space="PSUM"))
    # w1-only prefetch pool (w2 gets its own late pool)
    ffnwpool = ctx.enter_context(tc.tile_pool(name="ffnwp", bufs=1))
    bpool = ctx.enter_context(tc.tile_pool(name="bpool", bufs=1))
    act_stack = contextlib.ExitStack()
    actpool = act_stack.enter_context(tc.tile_pool(name="acts", bufs=1))
    xp_stack = contextlib.ExitStack()
    xppool = xp_stack.enter_context(tc.tile_pool(name="xpp", bufs=1))
    conv_stack = contextlib.ExitStack()
    convpool = conv_stack.enter_context(tc.tile_pool(name="convp", bufs=2))
    init_stack = contextlib.ExitStack()
    init_pool = init_stack.enter_context(tc.tile_pool(name="init", bufs=2))

    # ---- winT (Act ring) + xT chunks (SP ring) first: in_proj starts asap
    winT_sb = init_pool.tile([128, 8, 2 * 128 * NDT], BF16, name="winT")
    nc.scalar.dma_start(winT_sb[:, :, :],
                        winT.rearrange("(k p) m -> p k m", p=128))
    xTr0 = xT.rearrange("(k p) l -> p k l", p=128)
    xT_cs = []
    for c in range(L // 512):
        xT_c = init_pool.tile([128, 8, 512], BF16, name=f"xTc{c}")
        nc.sync.dma_start(xT_c[:, :, :], xTr0[:, :, c * 512:(c + 1) * 512])
        xT_cs.append(xT_c)

    # ---- small per-partition constants (SP ring, after winT) ----
    convw_sb, convb_sb, bdt_sb, acol_sb, dp_sb = [], [], [], [], []
    for dt in range(NDT):
        sl = slice(dt * 128, (dt + 1) * 128)
        t1 = singles.tile([128, DC], F32, name=f"cw{dt}")
        nc.sync.dma_start(t1[:, :], convw[sl, :])
        convw_sb.append(t1)
        t2 = singles.tile([128, 1], F32, name=f"cb{dt}")
        nc.sync.dma_start(t2[:, :], convb[sl, :])
        convb_sb.append(t2)
        t3 = singles.tile([128, 1], F32, name=f"bd{dt}")
        nc.sync.dma_start(t3[:, :], bdt[sl, :])
        bdt_sb.append(t3)
        t4 = singles.tile([128, DS], F32, name=f"ac{dt}")
        nc.sync.dma_start(t4[:, :], acol[sl, :])
        acol_sb.append(t4)
        t5 = singles.tile([128, 1], F32, name=f"dp{dt}")
        nc.sync.dma_start(t5[:, :], dpv[sl, :])
        dp_sb.append(t5)
    wxpT_sb = singles.tile([128, NDT, 96], BF16)
    nc.sync.dma_start(wxpT_sb[:, :, :], wxpT.rearrange("(k p) m -> p k m", p=128))
    wdtT_sb = singles.tile([DTR, DIC], BF16)
    nc.sync.dma_start(wdtT_sb[:, :], wdtT[:, :])
    ident_bf = singles.tile([128, 128], BF16)
    from concourse.masks import make_identity
    make_identity(nc, ident_bf[:, :])

    # ============ PHASE A: in_proj + conv + xproj, chunk-pipelined ============
    xTr = xT.rearrange("(k p) l -> p k l", p=128)
    ar_i = dram.tile([96, L], BF16)
    u_pad_c, u_act_c, zs, dbl_c = [], [], [], []
    for dt in range(NDT):
        u_pad_c.append([convpool.tile([128, 515], BF16, name=f"up{dt}_{c}")
                        for c in range(L // 512)])
        u_act_c.append([actpool.tile([128, 512], BF16, name=f"ua{dt}_{c}")
                        for c in range(L // 512)])
        zs.append(actpool.tile([128, L], BF16, name=f"zs{dt}"))
        nc.vector.memset(u_pad_c[dt][0][:, 0:3], 0.0)
    for c in range(L // 512):
        dbl_c.append(xppool.tile([96, 512], BF16, name=f"dbl{c}"))

    # pass 1: all u-tile matmuls (PE queue flows without xproj stalls)
    for c in range(L // 512):
        xT_c = xT_cs[c]
        for mt in range(NDT):
            ps = psum_small.tile([128, 512], F32, tag="psA")
            for k in range(8):
                nc.tensor.matmul(
                    ps[:, :],
                    winT_sb[:, k, mt * 128:(mt + 1) * 128],
                    xT_c[:, k, :],
                    start=(k == 0), stop=(k == 7))
            nc.scalar.copy(u_pad_c[mt][c][:, 3:515], ps[:, :])
            if c + 1 < L // 512:
                # halo: first 3 cols of next chunk = last 3 of this chunk
                nc.vector.tensor_copy(u_pad_c[mt][c + 1][:, 0:3],
                                      u_pad_c[mt][c][:, 512:515])
    # pass 2: conv chunks (each self-contained thanks to the halo)
    for c in range(L // 512):
        for dt in range(NDT):
            up = u_pad_c[dt][c]
            ca = convpool.tile([128, 512], F32, tag="cva")
            cb = convpool.tile([128, 512], F32, tag="cvb")
            nc.vector.tensor_scalar_mul(ca[:, :], up[:, 0:512],
                                        convw_sb[dt][:, 0:1])
            nc.vector.scalar_tensor_tensor(
                cb[:, :], up[:, 1:513], convw_sb[dt][:, 1:2],
                ca[:, :], op0=OP.mult, op1=OP.add)
            nc.vector.scalar_tensor_tensor(
                ca[:, :], up[:, 2:514], convw_sb[dt][:, 2:3],
                cb[:, :], op0=OP.mult, op1=OP.add)
            nc.vector.scalar_tensor_tensor(
                cb[:, :], up[:, 3:515], convw_sb[dt][:, 3:4],
                ca[:, :], op0=OP.mult, op1=OP.add)
            nc.scalar.activation(u_act_c[dt][c][:, :], cb[:, :], AF.Silu,
                                 bias=convb_sb[dt][:, 0:1])
    # pass 3: xproj partials + AR staging
    for c in range(L // 512):
        lo, hi = c * 512, (c + 1) * 512
        ps96 = psum_small.tile([96, 512], F32, tag="ps96")
        for kt in range(NDT):
            nc.tensor.matmul(
                ps96[:, :],
                wxpT_sb[:, kt, :],
                u_act_c[kt][c][:, :],
                start=(kt == 0), stop=(kt == NDT - 1))
        nc.scalar.copy(dbl_c[c][:, :], ps96[:, :])
        nc.sync.dma_start(ar_i[:, lo:hi], dbl_c[c][:, :])
    # z m-tiles: computed during the AllReduce window
    for c in range(L // 512):
        lo, hi = c * 512, (c + 1) * 512
        for mt in range(NDT, 2 * NDT):
            ps = psum_small.tile([128, 512], F32, tag="psA")
            for k in range(8):
                nc.tensor.matmul(
                    ps[:, :],
                    winT_sb[:, k, mt * 128:(mt + 1) * 128],
                    xT_cs[c][:, k, :],
                    start=(k == 0), stop=(k == 7))
            nc.scalar.activation(
                zs[mt - NDT][:, lo:hi], ps[:, :], AF.Silu)
    # ---- phase-B prefetches: after Phase A ring traffic
    w1qT_sb = ffnwpool.tile([128, DM // 128, DFF], FP8)
    nc.scalar.dma_start(w1qT_sb[:, :, :], w1qT.rearrange("(k p) j -> p k j", p=128))
    x_tok_sb = bpool.tile([128, NTT, DM], F32)
    nc.scalar.dma_start(x_tok_sb[:, :, :], x_tok.rearrange("(tt p) m -> p tt m", p=128))
    n1w_rep = bpool.tile([128, DM], F32)
    nc.scalar.dma_start(n1w_rep[:, :], _rep_ap(n1w[0:1, :]))
    n2w_rep = bpool.tile([128, DM], F32)
    nc.scalar.dma_start(n2w_rep[:, :], _rep_ap(n2w[0:1, :]))
    init_stack.close()
    conv_stack.close()
    if dbg:
        u32 = actpool.tile([128, L], F32, name="u32dbg")
        for c in range(L // 512):
            nc.vector.tensor_copy(u32[:, c * 512:(c + 1) * 512],
                                  u_act_c[0][c][:, :])
        nc.sync.dma_start(dbg_t["dbg_u"][:, :], u32[:, :])

    # ================= AllReduce (bf16) =================
    ar_o = dram.tile([96, L], BF16, addr_space="Shared")
    nc.gpsimd.collective_compute("AllReduce", OP.add, replica_groups=RG,
                                 ins=[ar_i.opt()], outs=[ar_o.opt()])
    # dt rows for the delta matmul; B/C rows are read straight from ar_o
    dt_bf = xppool.tile([DTR, L], BF16)
    nc.sync.dma_start(dt_bf[:, :], ar_o[0:DTR, :])
    if dbg:
        dbl_sb = xppool.tile([96, L], F32)
        nc.sync.dma_start(dbl_sb[:, :], ar_o[:, :])
        nc.sync.dma_start(dbg_t["dbg_dbl"][:, :], dbl_sb[:, :])
    bcb = ar_o[64:96, :]

    # ================= delta = softplus(wdt @ dt + bdt), bf16 ============
    delta = []
    for dt in range(NDT):
        dl = actpool.tile([128, L], BF16, name=f"delta{dt}")
        for c in range(L // 512):
            ps = psum_small.tile([128, 512], F32, tag="psA")
            nc.tensor.matmul(
                ps[:, :],
                wdtT_sb[:, dt * 128:(dt + 1) * 128],
                dt_bf[:, c * 512:(c + 1) * 512],
                start=True, stop=True)
            nc.scalar.activation(dl[:, c * 512:(c + 1) * 512], ps[:, :],
                                 AF.Exp, bias=bdt_sb[dt][:, 0:1])
        nc.scalar.activation(dl[:, :], dl[:, :], AF.Ln, bias=1.0)
        delta.append(dl)
    if dbg:
        d32 = actpool.tile([128, L], F32, name="d32dbg")
        nc.vector.tensor_copy(d32[:, :], delta[0][:, :])
        nc.sync.dma_start(dbg_t["dbg_delta"][:, :], d32[:, :])

    xp_stack.close()
    # delta*u in bf16 for the scan input product (both bf16 -> 2x DVE)
    du_bf = []
    for dt in range(NDT):
        db = actpool.tile([128, L], BF16, name=f"dubf{dt}")
        for c in range(L // 512):
            nc.vector.tensor_tensor(db[:, c * 512:(c + 1) * 512],
                                    delta[dt][:, c * 512:(c + 1) * 512],
                                    u_act_c[dt][c][:, :], op=OP.mult)
        du_bf.append(db)

    # ================= scan over n (16 states) =================
    psA_stack.close()
    yps_stack = contextlib.ExitStack()
    y_ps_pool = yps_stack.enter_context(
        tc.tile_pool(name="yps", bufs=1, space="PSUM"))
    y_ps = [y_ps_pool.tile([128, L], F32, name=f"yps{dt}") for dt in range(NDT)]

    scanp = act_stack.enter_context(tc.tile_pool(name="scanp", bufs=3))
    repp = act_stack.enter_context(tc.tile_pool(name="repp", bufs=3))
    a2a_i = [dram.tile([DI // 2, LT], BF16, name=f"a2ai{d}") for d in range(NDT)]
    a2a_o = [dram.tile([DI // 2, LT], BF16, name=f"a2ao{d}") for d in range(NDT)]
    yfull = bpool.tile([128, NDT, NCORES, LT], BF16, name="yfull")

    def emit_gate_a2a(dt):
        # gate for this dt, then its half-A2A (overlaps the other dt's scan)
        t1 = scanp.tile([128, L], BF16, name=f"yg{dt}", tag="dA")
        for c in range(L // 512):
            nc.vector.scalar_tensor_tensor(
                t1[:, c * 512:(c + 1) * 512], u_act_c[dt][c][:, :],
                dp_sb[dt][:, 0:1], y_ps[dt][:, c * 512:(c + 1) * 512],
                op0=OP.mult, op1=OP.add)
        yh = scanp.tile([128, L], BF16, name=f"yhat{dt}", tag="dBu")
        nc.vector.tensor_tensor(yh[:, :], t1[:, :], zs[dt][:, :], op=OP.mult)
        if dbg and dt == 0:
            yh32 = scanp.tile([128, L], F32, name="yh32", tag="yh32")
            nc.vector.tensor_copy(yh32[:, :], yh[:, :])
            nc.sync.dma_start(dbg_t["dbg_yhat"][:, :], yh32[:, :])
        # scatter my 128-ch rows into (8 token-blocks x 128) layout
        nc.sync.dma_start(
            a2a_i[dt].rearrange("(j c) t -> c j t", c=128)[:, :, :],
            yh.rearrange("c (j t) -> c j t", j=NCORES))

    for dt in range(NDT):
        for n in range(DS):
            if dt == 1 and n == 0:
                emit_gate_a2a(0)
            if dt == 1 and n == 4:
                # dt0's A2A collective emitted here: staging is complete by
                # now, so it never head-of-line blocks the Pool queue
                nc.gpsimd.collective_compute(
                    "AllToAll", OP.bypass, replica_groups=RG,
                    ins=[a2a_i[0].opt()], outs=[a2a_o[0].opt()])
                # prefetch dt0's half of yfull during the rest of dt1's scan
                nc.sync.dma_start(yfull[:, 0, :, :],
                                  a2a_o[0].rearrange("(j p) t -> p j t", p=128))
            brep = repp.tile([128, L], BF16, name=f"brep{dt}_{n}", tag="brep")
            nc.sync.dma_start(brep[:, :], _rep_ap(bcb[n:n + 1, :]))
            crep = repp.tile([128, L], BF16, name=f"crep{dt}_{n}", tag="crep")
            nc.scalar.dma_start(crep[:, :], _rep_ap(bcb[16 + n:17 + n, :]))
            dA = scanp.tile([128, L], BF16, name=f"dA{n}_{dt}", tag="dA")
            nc.scalar.activation(dA[:, :], delta[dt][:, :], AF.Exp,
                                 scale=acol_sb[dt][:, n:n + 1])
            dBu = scanp.tile([128, L], BF16, name=f"dBu{n}_{dt}", tag="dBu")
            nc.vector.tensor_tensor(dBu[:, :], du_bf[dt][:, :], brep[:, :],
                                    op=OP.mult)
            h = scanp.tile([128, L], BF16, name=f"h{n}_{dt}", tag="h")
            nc.vector.tensor_tensor_scan(h[:, :], dA[:, :], dBu[:, :], 0.0,
                                         OP.mult, OP.add)
            yt = scanp.tile([128, L], BF16, name=f"yt{n}_{dt}", tag="yt")
            eng_tt = nc.vector if n % 3 == 0 else nc.gpsimd
            eng_tt.tensor_tensor(yt[:, :], h[:, :], crep[:, :], op=OP.mult)
            for c in range(L // 512):
                nc.tensor.matmul(
                    y_ps[dt][:, c * 512:(c + 1) * 512],
                    ident_bf[:, :],
                    yt[:, c * 512:(c + 1) * 512],
                    start=(n == 0), stop=(n == DS - 1),
                    skip_group_check=True)
    emit_gate_a2a(1)
    nc.gpsimd.collective_compute("AllToAll", OP.bypass, replica_groups=RG,
                                 ins=[a2a_i[1].opt()], outs=[a2a_o[1].opt()])

    # ================= PHASE B (sequence-parallel, my LT tokens) ==========
    yps_stack.close()
    act_stack.close()
    psB = ctx.enter_context(tc.tile_pool(name="psB", bufs=2, space="PSUM"))
    psT = ctx.enter_context(tc.tile_pool(name="psT", bufs=2, space="PSUM"))
    scr = ctx.enter_context(tc.tile_pool(name="scr", bufs=1))
    bpB = ctx.enter_context(tc.tile_pool(name="bpB", bufs=1))
    w2pool = ctx.enter_context(tc.tile_pool(name="w2p", bufs=1))
    # wout + w2 loads: start as soon as scan-era SBUF frees; overlap A2A wait
    wout_stack = contextlib.ExitStack()
    woutpool = wout_stack.enter_context(tc.tile_pool(name="woutp", bufs=1))
    woutT_sb = woutpool.tile([128, DI // 128, DM], BF16)
    nc.sync.dma_start(woutT_sb[:, :, :], woutT.rearrange("(k p) m -> p k m", p=128))
    w2qT_sb = w2pool.tile([128, DFF // 128, DM], FP8)
    nc.scalar.dma_start(w2qT_sb[:, :, :], w2qT.rearrange("(k p) m -> p k m", p=128))

    x1_l, scl1_l, xqT_l, fqT_l, scl2_l = [], [], [], [], []

    # ---- out_proj + rmsnorm1 + quant1 ----
    with tc.tile_pool(name="oproj", bufs=1) as opool:
        nc.sync.dma_start(yfull[:, 1, :, :],
                          a2a_o[1].rearrange("(j p) t -> p j t", p=128))
        for tt in range(NTT):
            hps = psB.tile([128, DM], F32, tag="hps")
            for c in range(DM // 512):
                kk = 0
                for d in range(NDT):
                    for j in range(NCORES):
                        nc.tensor.matmul(
                            hps[:, c * 512:(c + 1) * 512],
                            yfull[:, d, j, tt * 128:(tt + 1) * 128],
                            woutT_sb[:, j * NDT + d, c * 512:(c + 1) * 512],
                            start=(kk == 0), stop=(kk == DI // 128 - 1))
                        kk += 1
            s = scr.tile([128, DM], F32, name=f"s{tt}", tag="sscr")
            nc.vector.tensor_tensor(s[:, :], x_tok_sb[:, tt, :], hps[:, :], op=OP.add)
            sq = scr.tile([128, DM], F32, name=f"sq{tt}", tag="sqscr")
            ssum = bpB.tile([128, 1], F32, name=f"ssum{tt}", tag="ssum")
            nc.scalar.activation(sq[:, :], s[:, :], AF.Square, accum_out=ssum[:, 0:1])
            v = bpB.tile([128, 1], F32, name=f"v{tt}", tag=f"v{tt}")
            nc.vector.tensor_scalar(v[:, :], ssum[:, :], 1.0 / DM, EPS,
                                    op0=OP.mult, op1=OP.add)
            nc.scalar.activation(v[:, :], v[:, :], AF.Sqrt)
            nc.vector.reciprocal(v[:, :], v[:, :])
            x1 = bpB.tile([128, DM], F32, name=f"x1_{tt}", tag=f"x1_{tt}")
            nc.vector.scalar_tensor_tensor(x1[:, :], s[:, :], v[:, 0:1],
                                           n1w_rep[:, :], op0=OP.mult, op1=OP.mult)
            x1_l.append(x1)
            if dbg and tt == 0:
                nc.sync.dma_start(dbg_t["dbg_x1"][:, :], x1[:, :])
            amax = bpB.tile([128, 1], F32, name=f"am{tt}", tag="am")
            nc.vector.tensor_reduce(amax[:, :], x1[:, :], axis=mybir.AxisListType.X,
                                    op=OP.max, apply_absolute_value=True)
            nc.vector.tensor_scalar(amax[:, :], amax[:, :], 1e-5, None, op0=OP.max)
            sc = bpB.tile([128, 1], F32, name=f"sc{tt}", tag="sc")
            nc.vector.reciprocal(sc[:, :], amax[:, :])
            nc.vector.tensor_scalar(sc[:, :], sc[:, :], 127.0, None, op0=OP.mult)
            scl1 = bpB.tile([128, 1], F32, name=f"scl1_{tt}", tag=f"scl1_{tt}")
            nc.vector.tensor_scalar(scl1[:, :], amax[:, :], g1 / 127.0, None,
                                    op0=OP.mult)
            scl1_l.append(scl1)
            q = scr.tile([128, DM], BF16, name=f"q{tt}", tag="qscr")
            nc.vector.tensor_scalar(q[:, :], x1[:, :], sc[:, 0:1], None, op0=OP.mult)
            nc.vector.tensor_scalar(q[:, :], q[:, :], MAGIC_BF, MAGIC_BF,
                                    op0=OP.add, op1=OP.subtract)
            xq = scr.tile([128, DM], BF16, name=f"xq{tt}", tag="xqscr")
            nc.vector.tensor_scalar(xq[:, :], q[:, :], 127.0, -128.0,
                                    op0=OP.min, op1=OP.max)
            # transpose via PE (8 x [128,128])
            xqT = bpB.tile([128, DM // 128, 128], BF16, name=f"xqT{tt}",
                           tag=f"xqT{tt}")
            for k in range(DM // 128):
                tp = psT.tile([128, 128], BF16, tag="tp")
                nc.tensor.transpose(tp[:, :], xq[:, k * 128:(k + 1) * 128],
                                    ident_bf[:, :])
                nc.scalar.copy(xqT[:, k, :], tp[:, :])
            xqT_l.append(xqT)
    wout_stack.close()

    # ---- FFN mm1 + gelu + quant2 ----
    with tc.tile_pool(name="ffn1", bufs=1) as f1pool:
        for tt in range(NTT):
            f_sb = f1pool.tile([128, DFF], BF16, name=f"f{tt}", tag=f"f{tt}")
            for jc in range(DFF // 512):
                fps = psB.tile([128, 512], F32, tag="mmps")
                for k in range(DM // 128):
                    nc.tensor.matmul(
                        fps[:, :], xqT_l[tt][:, k, :],
                        w1qT_sb[:, k, jc * 512:(jc + 1) * 512],
                        start=(k == 0), stop=(k == DM // 128 - 1))
                nc.scalar.activation(f_sb[:, jc * 512:(jc + 1) * 512], fps[:, :],
                                     AF.Gelu_apprx_tanh, scale=scl1_l[tt][:, 0:1])
            if dbg and tt == 0:
                f32dbg = f1pool.tile([128, DFF], F32, name="f32dbg", tag="f32dbg")
                nc.vector.tensor_copy(f32dbg[:, :], f_sb[:, :])
                nc.sync.dma_start(dbg_t["dbg_f"][:, :], f32dbg[:, :])
            amax2 = bpB.tile([128, 1], F32, name=f"am2{tt}", tag="am2")
            nc.vector.tensor_reduce(amax2[:, :], f_sb[:, :],
                                    axis=mybir.AxisListType.X,
                                    op=OP.max, apply_absolute_value=True)
            nc.vector.tensor_scalar(amax2[:, :], amax2[:, :], 1e-5, None, op0=OP.max)
            sc2 = bpB.tile([128, 1], F32, name=f"sc2{tt}", tag="sc2")
            nc.vector.reciprocal(sc2[:, :], amax2[:, :])
            nc.vector.tensor_scalar(sc2[:, :], sc2[:, :], 127.0, None, op0=OP.mult)
            scl2 = bpB.tile([128, 1], F32, name=f"scl2_{tt}", tag=f"scl2_{tt}")
            nc.vector.tensor_scalar(scl2[:, :], amax2[:, :], g2 / 127.0, None,
                                    op0=OP.mult)
            scl2_l.append(scl2)
            q2 = f1pool.tile([128, DFF], BF16, name=f"q2{tt}", tag="q2scr")
            nc.vector.tensor_scalar(q2[:, :], f_sb[:, :], sc2[:, 0:1], None,
                                    op0=OP.mult)
            nc.vector.tensor_scalar(q2[:, :], q2[:, :], MAGIC_BF, MAGIC_BF,
                                    op0=OP.add, op1=OP.subtract)
            fq = f1pool.tile([128, DFF], BF16, name=f"fq{tt}", tag="fqscr")
            nc.vector.tensor_scalar(fq[:, :], q2[:, :], 127.0, -128.0,
                                    op0=OP.min, op1=OP.max)
            fqT = bpB.tile([128, DFF // 128, 128], BF16, name=f"fqT{tt}",
                           tag=f"fqT{tt}")
            for k in range(DFF // 128):
                tp = psT.tile([128, 128], BF16, tag="tp")
                nc.tensor.transpose(tp[:, :], fq[:, k * 128:(k + 1) * 128],
                                    ident_bf[:, :])
                nc.scalar.copy(fqT[:, k, :], tp[:, :])
            fqT_l.append(fqT)

    # ---- FFN mm2 + residual + rmsnorm2 ----
    with tc.tile_pool(name="ffn2", bufs=1) as f2pool:
        for tt in range(NTT):
            o2 = f2pool.tile([128, DM], F32, name=f"o2{tt}", tag=f"o2{tt}")
            for mc in range(DM // 512):
                ops_ = psB.tile([128, 512], F32, tag="mmps")
                for k in range(DFF // 128):
                    nc.tensor.matmul(
                        ops_[:, :], fqT_l[tt][:, k, :],
                        w2qT_sb[:, k, mc * 512:(mc + 1) * 512],
                        start=(k == 0), stop=(k == DFF // 128 - 1))
                nc.vector.scalar_tensor_tensor(
                    o2[:, mc * 512:(mc + 1) * 512], ops_[:, :], scl2_l[tt][:, 0:1],
                    x1_l[tt][:, mc * 512:(mc + 1) * 512], op0=OP.mult, op1=OP.add)
            sq2 = f2pool.tile([128, DM], F32, name=f"sq2{tt}", tag="sq2scr")
            ssum2 = f2pool.tile([128, 1], F32, name=f"ssum2{tt}", tag="ssum2")
            nc.scalar.activation(sq2[:, :], o2[:, :], AF.Square,
                                 accum_out=ssum2[:, 0:1])
            v2 = f2pool.tile([128, 1], F32, name=f"v2{tt}", tag=f"v2{tt}")
            nc.vector.tensor_scalar(v2[:, :], ssum2[:, :], 1.0 / DM, EPS,
                                    op0=OP.mult, op1=OP.add)
            nc.scalar.activation(v2[:, :], v2[:, :], AF.Sqrt)
            nc.vector.reciprocal(v2[:, :], v2[:, :])
            ot = f2pool.tile([128, DM], F32, name=f"ot{tt}", tag="otscr")
            nc.vector.scalar_tensor_tensor(ot[:, :], o2[:, :], v2[:, 0:1],
                                           n2w_rep[:, :], op0=OP.mult, op1=OP.mult)
            nc.sync.dma_start(out_t[tt * 128:(tt + 1) * 128, :], ot[:, :])


def build_nc(g1, g2, dbg=False):
    from contextlib import ExitStack
    nc = bacc.Bacc("TRN2", target_bir_lowering=False, debug=False,
                   num_devices=NCORES)
    with ExitStack() as ctx:
        tc = ctx.enter_context(tile.TileContext(nc))
        _emit(nc, tc, ctx, g1, g2, dbg)
    nc.compile()
    return nc


def host_prep(inputs):
    bf = ml_dtypes.bfloat16
    f8 = ml_dtypes.float8_e4m3
    x = np.asarray(inputs["x"], np.float32)
    x2d = x.reshape(L, DM)
    w_in = np.asarray(inputs["w_in"], np.float32)
    conv_w = np.asarray(inputs["conv_w"], np.float32)
    conv_b = np.asarray(inputs["conv_b"], np.float32)
    w_xproj = np.asarray(inputs["w_xproj"], np.float32)
    w_dt = np.asarray(inputs["w_dt"], np.float32)
    b_dt = np.asarray(inputs["b_dt"], np.float32)
    A_log = np.asarray(inputs["A_log"], np.float32)
    Dp = np.asarray(inputs["Dp"], np.float32)
    w_out = np.asarray(inputs["w_out"], np.float32)
    n1 = np.asarray(inputs["norm1_w"], np.float32)
    n2 = np.asarray(inputs["norm2_w"], np.float32)
    w1 = np.asarray(inputs["ffn_w1"], np.float32)
    w2 = np.asarray(inputs["ffn_w2"], np.float32)
    b1 = np.asarray(inputs["ffn_b1"], np.float32)
    b2 = np.asarray(inputs["ffn_b2"], np.float32)
    assert np.all(b1 == 0.0) and np.all(b2 == 0.0), "nonzero ffn bias unsupported"

    g1 = float(np.maximum(np.mean(np.abs(w1), dtype=np.float32), 1e-5))
    g2 = float(np.maximum(np.mean(np.abs(w2), dtype=np.float32), 1e-5))
    w1q = np.clip(np.rint(w1 / g1), -1.0, 1.0).astype(np.float32)
    w2q = np.clip(np.rint(w2 / g2), -1.0, 1.0).astype(np.float32)

    xT_bf = np.ascontiguousarray(x2d.T).astype(bf)
    woutT_bf = np.ascontiguousarray(w_out.T).astype(bf)
    w1qT_f8 = np.ascontiguousarray(w1q.T).astype(f8)
    w2qT_f8 = np.ascontiguousarray(w2q.T).astype(f8)
    n1r = np.ascontiguousarray(n1.reshape(1, DM))
    n2r = np.ascontiguousarray(n2.reshape(1, DM))
    A = -np.exp(A_log)

    in_maps = []
    for c in range(NCORES):
        ch = slice(c * DIC, (c + 1) * DIC)
        w_sel = np.concatenate([w_in[c * DIC:(c + 1) * DIC],
                                w_in[DI + c * DIC:DI + (c + 1) * DIC]], axis=0)
        in_maps.append({
            "xT": xT_bf,
            "x_tok": np.ascontiguousarray(x2d[c * LT:(c + 1) * LT]),
            "winT": np.ascontiguousarray(w_sel.T).astype(bf),
            "convw": np.ascontiguousarray(conv_w[ch, 0, :]),
            "convb": np.ascontiguousarray(conv_b[ch].reshape(-1, 1)),
            "wxpT": np.ascontiguousarray(w_xproj[:, ch].T).astype(bf),
            "wdtT": np.ascontiguousarray(w_dt[ch, :].T).astype(bf),
            "bdt": np.ascontiguousarray(b_dt[ch].reshape(-1, 1)),
            "acol": np.ascontiguousarray(A[ch, :]),
            "dpv": np.ascontiguousarray(Dp[ch].reshape(-1, 1)),
            "woutT": woutT_bf,
            "n1w": n1r,
            "n2w": n2r,
            "w1qT": w1qT_f8,
            "w2qT": w2qT_f8,
        })
    return in_maps, g1, g2


def kernel(**inputs) -> np.ndarray:
    in_maps, g1, g2 = host_prep(inputs)
    key = (round(g1, 10), round(g2, 10))
    if key not in _NC_CACHE:
        _NC_CACHE[key] = build_nc(g1, g2)
    nc = _NC_CACHE[key]
    res = run_bass_kernel_spmd(nc, in_maps, core_ids=list(range(NCORES)))
    out = np.concatenate([res.results[c]["out"] for c in range(NCORES)], axis=0)
    return np.ascontiguousarray(out.reshape(1, L, DM).astype(np.float32))


# revision 5
# speedup vs baseline: 53.7861x; 1.0034x over previous
"""Trainium2 Bass kernel v2 for nn_DecoderLayer (Mamba block + BitNet FFN).

Sharding: channel-parallel mamba (256 ch/core) -> AllReduce (xproj rows) ->
DVE tensor_tensor_scan over (d,n) lanes -> AllToAll (d-shard -> t-shard) ->
sequence-parallel out_proj + rmsnorm + BitNet FFN (fp8 ternary weights) ->
each core emits its 256-token slice.

v2: fp8e4 FFN weights (halved HBM/tunnel bytes), w1 prefetched at t0 and
wout/w2 loads overlapped with scan/A2A, bf16 datapath for 2x DVE modes,
B/C replication split across both HWDGE rings, native Softplus, PE
transposes instead of DMA transposes, chunked xT streaming.
"""
import numpy as np
import ml_dtypes

try:
    import jax
    jax.config.update("jax_compilation_cache_dir", "/root/jaxcache")
    jax.config.update("jax_persistent_cache_min_compile_time_secs", 1.0)
except Exception:
    pass

import concourse.bass as bass
import concourse.mybir as mybir
import concourse.tile as tile
from concourse import bacc
from concourse.bass_utils import run_bass_kernel_spmd

BF16 = mybir.dt.bfloat16
F32 = mybir.dt.float32
FP8 = mybir.dt.float8e4
AF = mybir.ActivationFunctionType
OP = mybir.AluOpType

L, DM, DI, DS, DC, DTR, DFF = 2048, 1024, 2048, 16, 4, 64, 4096
EPS = 1e-6
NCORES = 8
DIC = DI // NCORES   # 256 channels per core
NDT = DIC // 128     # 2 d-tiles
LT = L // NCORES     # 256 tokens per core
NTT = LT // 128      # 2 token-tiles
MAGIC = 12582912.0   # 1.5*2^23: x+M-M == rint(x) for |x|<2^22 (f32)
MAGIC_BF = 384.0     # 1.5*2^8: bf16 magic for ints in [-128,127]

_NC_CACHE = {}


def _rep_ap(src):
    """Broadcast a 1-row DRAM slice across 128 partitions."""
    return bass.AP(tensor=src.tensor, offset=src.offset,
                   ap=[[0, 128]] + [list(p) for p in src.ap[1:]])


def _emit(nc, tc, ctx, g1, g2, dbg):
    import contextlib
    RG = [list(range(NCORES))]

    xT = nc.dram_tensor("xT", [DM, L], BF16, kind="ExternalInput")
    x_tok = nc.dram_tensor("x_tok", [LT, DM], F32, kind="ExternalInput")
    winT = nc.dram_tensor("winT", [DM, 2 * 128 * NDT], BF16, kind="ExternalInput")
    convw = nc.dram_tensor("convw", [DIC, DC], F32, kind="ExternalInput")
    convb = nc.dram_tensor("convb", [DIC, 1], F32, kind="ExternalInput")
    wxpT = nc.dram_tensor("wxpT", [DIC, 96], BF16, kind="ExternalInput")
    wdtT = nc.dram_tensor("wdtT", [DTR, DIC], BF16, kind="ExternalInput")
    bdt = nc.dram_tensor("bdt", [DIC, 1], F32, kind="ExternalInput")
    acol = nc.dram_tensor("acol", [DIC, DS], F32, kind="ExternalInput")
    dpv = nc.dram_tensor("dpv", [DIC, 1], F32, kind="ExternalInput")
    woutT = nc.dram_tensor("woutT", [DI, DM], BF16, kind="ExternalInput")
    n1w = nc.dram_tensor("n1w", [1, DM], F32, kind="ExternalInput")
    n2w = nc.dram_tensor("n2w", [1, DM], F32, kind="ExternalInput")
    w1qT = nc.dram_tensor("w1qT", [DM, DFF], FP8, kind="ExternalInput")
    w2qT = nc.dram_tensor("w2qT", [DFF, DM], FP8, kind="ExternalInput")
    out_t = nc.dram_tensor("out", [LT, DM], F32, kind="ExternalOutput")
    dbg_t = {}
    if dbg:
        dbg_t["dbg_u"] = nc.dram_tensor("dbg_u", [128, L], F32, kind="ExternalOutput")
        dbg_t["dbg_delta"] = nc.dram_tensor("dbg_delta", [128, L], F32, kind="ExternalOutput")
        dbg_t["dbg_dbl"] = nc.dram_tensor("dbg_dbl", [96, L], F32, kind="ExternalOutput")
        dbg_t["dbg_yhat"] = nc.dram_tensor("dbg_yhat", [128, L], F32, kind="ExternalOutput")
        dbg_t["dbg_x1"] = nc.dram_tensor("dbg_x1", [128, DM], F32, kind="ExternalOutput")
        dbg_t["dbg_f"] = nc.dram_tensor("dbg_f", [128, DFF], F32, kind="ExternalOutput")

    singles = ctx.enter_context(tc.tile_pool(name="singles", bufs=1))
    dram = ctx.enter_context(tc.tile_pool(name="dram", bufs=1, space="DRAM"))
    psA_stack = contextlib.ExitStack()
    psum_small = psA_stack.enter_context(
        tc.tile_pool(name="psA", bufs=3, space="PSUM"))
    # w1-only prefetch pool (w2 gets its own late pool)
    ffnwpool = ctx.enter_context(tc.tile_pool(name="ffnwp", bufs=1))
    bpool = ctx.enter_context(tc.tile_pool(name="bpool", bufs=1))
    act_stack = contextlib.ExitStack()
    actpool = act_stack.enter_context(tc.tile_pool(name="acts", bufs=1))
    xp_stack = contextlib.ExitStack()
    xppool = xp_stack.enter_context(tc.tile_pool(name="xpp", bufs=1))
    conv_stack = contextlib.ExitStack()
    convpool = conv_stack.enter_context(tc.tile_pool(name="convp", bufs=2))
    init_stack = contextlib.ExitStack()
    init_pool = init_stack.enter_context(tc.tile_pool(name="init", bufs=2))

    # ---- winT (Act ring) + xT chunks (SP ring) first: in_proj starts asap
    winT_sb = init_pool.tile([128, 8, 2 * 128 * NDT], BF16, name="winT")
    nc.scalar.dma_start(winT_sb[:, :, :],
                        winT.rearrange("(k p) m -> p k m", p=128))
    xTr0 = xT.rearrange("(k p) l -> p k l", p=128)
    xT_cs = []
    for c in range(L // 512):
        xT_c = init_pool.tile([128, 8, 512], BF16, name=f"xTc{c}")
        nc.sync.dma_start(xT_c[:, :, :], xTr0[:, :, c * 512:(c + 1) * 512])
        xT_cs.append(xT_c)

    # ---- small per-partition constants (SP ring, after winT) ----
    convw_sb, convb_sb, bdt_sb, acol_sb, dp_sb = [], [], [], [], []
    for dt in range(NDT):
        sl = slice(dt * 128, (dt + 1) * 128)
        t1 = singles.tile([128, DC], F32, name=f"cw{dt}")
        nc.sync.dma_start(t1[:, :], convw[sl, :])
        convw_sb.append(t1)
        t2 = singles.tile([128, 1], F32, name=f"cb{dt}")
        nc.sync.dma_start(t2[:, :], convb[sl, :])
        convb_sb.append(t2)
        t3 = singles.tile([128, 1], F32, name=f"bd{dt}")
        nc.sync.dma_start(t3[:, :], bdt[sl, :])
        bdt_sb.append(t3)
        t4 = singles.tile([128, DS], F32, name=f"ac{dt}")
        nc.sync.dma_start(t4[:, :], acol[sl, :])
        acol_sb.append(t4)
        t5 = singles.tile([128, 1], F32, name=f"dp{dt}")
        nc.sync.dma_start(t5[:, :], dpv[sl, :])
        dp_sb.append(t5)
    wxpT_sb = singles.tile([128, NDT, 96], BF16)
    nc.sync.dma_start(wxpT_sb[:, :, :], wxpT.rearrange("(k p) m -> p k m", p=128))
    wdtT_sb = singles.tile([DTR, DIC], BF16)
    nc.sync.dma_start(wdtT_sb[:, :], wdtT[:, :])
    ident_bf = singles.tile([128, 128], BF16)
    from concourse.masks import make_identity
    make_identity(nc, ident_bf[:, :])

    # ============ PHASE A: in_proj + conv + xproj, chunk-pipelined ============
    xTr = xT.rearrange("(k p) l -> p k l", p=128)
    ar_i = dram.tile([96, L], BF16)
    u_pad_c, u_act_c, zs, dbl_c = [], [], [], []
    for dt in range(NDT):
        u_pad_c.append([convpool.tile([128, 515], BF16, name=f"up{dt}_{c}")
                        for c in range(L // 512)])
        u_act_c.append([actpool.tile([128, 512], BF16, name=f"ua{dt}_{c}")
                        for c in range(L // 512)])
        zs.append(actpool.tile([128, L], BF16, name=f"zs{dt}"))
        nc.vector.memset(u_pad_c[dt][0][:, 0:3], 0.0)
    for c in range(L // 512):
        dbl_c.append(xppool.tile([96, 512], BF16, name=f"dbl{c}"))

    # pass 1: all u-tile matmuls (PE queue flows without xproj stalls)
    for c in range(L // 512):
        xT_c = xT_cs[c]
        for mt in range(NDT):
            ps = psum_small.tile([128, 512], F32, tag="psA")
            for k in range(8):
                nc.tensor.matmul(
                    ps[:, :],
                    winT_sb[:, k, mt * 128:(mt + 1) * 128],
                    xT_c[:, k, :],
                    start=(k == 0), stop=(k == 7))
            nc.scalar.copy(u_pad_c[mt][c][:, 3:515], ps[:, :])
            if c + 1 < L // 512:
                # halo: first 3 cols of next chunk = last 3 of this chunk
                nc.vector.tensor_copy(u_pad_c[mt][c + 1][:, 0:3],
                                      u_pad_c[mt][c][:, 512:515])
    # pass 2: conv chunks (each self-contained thanks to the halo)
    for c in range(L // 512):
        for dt in range(NDT):
            up = u_pad_c[dt][c]
            ca = convpool.tile([128, 512], F32, tag="cva")
            cb = convpool.tile([128, 512], F32, tag="cvb")
            nc.vector.tensor_scalar_mul(ca[:, :], up[:, 0:512],
                                        convw_sb[dt][:, 0:1])
            nc.vector.scalar_tensor_tensor(
                cb[:, :], up[:, 1:513], convw_sb[dt][:, 1:2],
                ca[:, :], op0=OP.mult, op1=OP.add)
            nc.vector.scalar_tensor_tensor(
                ca[:, :], up[:, 2:514], convw_sb[dt][:, 2:3],
                cb[:, :], op0=OP.mult, op1=OP.add)
            nc.vector.scalar_tensor_tensor(
                cb[:, :], up[:, 3:515], convw_sb[dt][:, 3:4],
                ca[:, :], op0=OP.mult, op1=OP.add)
            nc.scalar.activation(u_act_c[dt][c][:, :], cb[:, :], AF.Silu,
                                 bias=convb_sb[dt][:, 0:1])
    # pass 3: xproj partials + AR staging
    for c in range(L // 512):
        lo, hi = c * 512, (c + 1) * 512
        ps96 = psum_small.tile([96, 512], F32, tag="ps96")
        for kt in range(NDT):
            nc.tensor.matmul(
                ps96[:, :],
                wxpT_sb[:, kt, :],
                u_act_c[kt][c][:, :],
                start=(kt == 0), stop=(kt == NDT - 1))
        nc.scalar.copy(dbl_c[c][:, :], ps96[:, :])
        nc.sync.dma_start(ar_i[:, lo:hi], dbl_c[c][:, :])
    # z m-tiles: computed during the AllReduce window
    for c in range(L // 512):
        lo, hi = c * 512, (c + 1) * 512
        for mt in range(NDT, 2 * NDT):
            ps = psum_small.tile([128, 512], F32, tag="psA")
            for k in range(8):
                nc.tensor.matmul(
                    ps[:, :],
                    winT_sb[:, k, mt * 128:(mt + 1) * 128],
                    xT_cs[c][:, k, :],
                    start=(k == 0), stop=(k == 7))
            nc.scalar.activation(
                zs[mt - NDT][:, lo:hi], ps[:, :], AF.Silu)
    # ---- phase-B prefetches: after Phase A ring traffic
    w1qT_sb = ffnwpool.tile([128, DM // 128, DFF], FP8)
    nc.scalar.dma_start(w1qT_sb[:, :, :], w1qT.rearrange("(k p) j -> p k j", p=128))
    x_tok_sb = bpool.tile([128, NTT, DM], F32)
    nc.scalar.dma_start(x_tok_sb[:, :, :], x_tok.rearrange("(tt p) m -> p tt m", p=128))
    n1w_rep = bpool.tile([128, DM], F32)
    nc.scalar.dma_start(n1w_rep[:, :], _rep_ap(n1w[0:1, :]))
    n2w_rep = bpool.tile([128, DM], F32)
    nc.scalar.dma_start(n2w_rep[:, :], _rep_ap(n2w[0:1, :]))
    init_stack.close()
    conv_stack.close()
    if dbg:
        u32 = actpool.tile([128, L], F32, name="u32dbg")
        for c in range(L // 512):
            nc.vector.tensor_copy(u32[:, c * 512:(c + 1) * 512],
                                  u_act_c[0][c][:, :])
        nc.sync.dma_start(dbg_t["dbg_u"][:, :], u32[:, :])

    # ================= AllReduce (bf16) =================
    ar_o = dram.tile([96, L], BF16, addr_space="Shared")
    nc.gpsimd.collective_compute("AllReduce", OP.add, replica_groups=RG,
                                 ins=[ar_i.opt()], outs=[ar_o.opt()])
    # dt rows for the delta matmul; B/C rows are read straight from ar_o
    dt_bf = xppool.tile([DTR, L], BF16)
    nc.sync.dma_start(dt_bf[:, :], ar_o[0:DTR, :])
    if dbg:
        dbl_sb = xppool.tile([96, L], F32)
        nc.sync.dma_start(dbl_sb[:, :], ar_o[:, :])
        nc.sync.dma_start(dbg_t["dbg_dbl"][:, :], dbl_sb[:, :])
    bcb = ar_o[64:96, :]

    # ================= delta = softplus(wdt @ dt + bdt), bf16 ============
    delta = [None, None]
    du_bf = [None, None]
    for dt in range(NDT):
        dl = actpool.tile([128, L], BF16, name=f"delta{dt}")
        for c in range(L // 512):
            ps = psum_small.tile([128, 512], F32, tag="psA")
            nc.tensor.matmul(
                ps[:, :],
                wdtT_sb[:, dt * 128:(dt + 1) * 128],
                dt_bf[:, c * 512:(c + 1) * 512],
                start=True, stop=True)
            nc.scalar.activation(dl[:, c * 512:(c + 1) * 512], ps[:, :],
                                 AF.Exp, bias=bdt_sb[dt][:, 0:1])
        nc.scalar.activation(dl[:, :], dl[:, :], AF.Ln, bias=1.0)
        delta[dt] = dl
        # delta*u for this dt immediately (dt0's scan starts without waiting
        # for dt1's delta)
        db = actpool.tile([128, L], BF16, name=f"dubf{dt}")
        for c in range(L // 512):
            nc.vector.tensor_tensor(db[:, c * 512:(c + 1) * 512],
                                    delta[dt][:, c * 512:(c + 1) * 512],
                                    u_act_c[dt][c][:, :], op=OP.mult)
        du_bf[dt] = db
    if dbg:
        d32 = actpool.tile([128, L], F32, name="d32dbg")
        nc.vector.tensor_copy(d32[:, :], delta[0][:, :])
        nc.sync.dma_start(dbg_t["dbg_delta"][:, :], d32[:, :])

    xp_stack.close()

    # ================= scan over n (16 states) =================
    psA_stack.close()
    yps_stack = contextlib.ExitStack()
    y_ps_pool = yps_stack.enter_context(
        tc.tile_pool(name="yps", bufs=1, space="PSUM"))
    y_ps = [y_ps_pool.tile([128, L], F32, name=f"yps{dt}") for dt in range(NDT)]

    scanp = act_stack.enter_context(tc.tile_pool(name="scanp", bufs=3))
    repp = act_stack.enter_context(tc.tile_pool(name="repp", bufs=4))
    a2a_i = [dram.tile([DI // 2, LT], BF16, name=f"a2ai{d}") for d in range(NDT)]
    a2a_o = [dram.tile([DI // 2, LT], BF16, name=f"a2ao{d}") for d in range(NDT)]
    yfull = bpool.tile([128, NDT, NCORES, LT], BF16, name="yfull")

    def emit_gate_a2a(dt):
        # gate for this dt, then its half-A2A (overlaps the other dt's scan)
        t1 = scanp.tile([128, L], BF16, name=f"yg{dt}", tag="dA")
        for c in range(L // 512):
            nc.vector.scalar_tensor_tensor(
                t1[:, c * 512:(c + 1) * 512], u_act_c[dt][c][:, :],
                dp_sb[dt][:, 0:1], y_ps[dt][:, c * 512:(c + 1) * 512],
                op0=OP.mult, op1=OP.add)
        yh = scanp.tile([128, L], BF16, name=f"yhat{dt}", tag="dBu")
        nc.vector.tensor_tensor(yh[:, :], t1[:, :], zs[dt][:, :], op=OP.mult)
        if dbg and dt == 0:
            yh32 = scanp.tile([128, L], F32, name="yh32", tag="yh32")
            nc.vector.tensor_copy(yh32[:, :], yh[:, :])
            nc.sync.dma_start(dbg_t["dbg_yhat"][:, :], yh32[:, :])
        # scatter my 128-ch rows into (8 token-blocks x 128) layout
        nc.sync.dma_start(
            a2a_i[dt].rearrange("(j c) t -> c j t", c=128)[:, :, :],
            yh.rearrange("c (j t) -> c j t", j=NCORES))

    for dt in range(NDT):
        for n in range(DS):
            if dt == 1 and n == 0:
                emit_gate_a2a(0)
            if dt == 1 and n == 4:
                # dt0's A2A collective emitted here: staging is complete by
                # now, so it never head-of-line blocks the Pool queue
                nc.gpsimd.collective_compute(
                    "AllToAll", OP.bypass, replica_groups=RG,
                    ins=[a2a_i[0].opt()], outs=[a2a_o[0].opt()])
                # prefetch dt0's half of yfull during the rest of dt1's scan
                nc.sync.dma_start(yfull[:, 0, :, :],
                                  a2a_o[0].rearrange("(j p) t -> p j t", p=128))
            brep = repp.tile([128, L], BF16, name=f"brep{dt}_{n}", tag="brep")
            nc.sync.dma_start(brep[:, :], _rep_ap(bcb[n:n + 1, :]))
            crep = repp.tile([128, L], BF16, name=f"crep{dt}_{n}", tag="crep")
            nc.scalar.dma_start(crep[:, :], _rep_ap(bcb[16 + n:17 + n, :]))
            dA = scanp.tile([128, L], BF16, name=f"dA{n}_{dt}", tag="dA")
            nc.scalar.activation(dA[:, :], delta[dt][:, :], AF.Exp,
                                 scale=acol_sb[dt][:, n:n + 1])
            dBu = scanp.tile([128, L], BF16, name=f"dBu{n}_{dt}", tag="dBu")
            nc.vector.tensor_tensor(dBu[:, :], du_bf[dt][:, :], brep[:, :],
                                    op=OP.mult)
            h = scanp.tile([128, L], BF16, name=f"h{n}_{dt}", tag="h")
            nc.vector.tensor_tensor_scan(h[:, :], dA[:, :], dBu[:, :], 0.0,
                                         OP.mult, OP.add)
            yt = scanp.tile([128, L], BF16, name=f"yt{n}_{dt}", tag="yt")
            eng_tt = nc.vector if n % 3 == 0 else nc.gpsimd
            eng_tt.tensor_tensor(yt[:, :], h[:, :], crep[:, :], op=OP.mult)
            for c in range(L // 512):
                nc.tensor.matmul(
                    y_ps[dt][:, c * 512:(c + 1) * 512],
                    ident_bf[:, :],
                    yt[:, c * 512:(c + 1) * 512],
                    start=(n == 0), stop=(n == DS - 1),
                    skip_group_check=True)
    emit_gate_a2a(1)
    nc.gpsimd.collective_compute("AllToAll", OP.bypass, replica_groups=RG,
                                 ins=[a2a_i[1].opt()], outs=[a2a_o[1].opt()])

    # ================= PHASE B (sequence-parallel, my LT tokens) ==========
    yps_stack.close()
    act_stack.close()
    psB = ctx.enter_context(tc.tile_pool(name="psB", bufs=2, space="PSUM"))
    psT = ctx.enter_context(tc.tile_pool(name="psT", bufs=2, space="PSUM"))
    scr = ctx.enter_context(tc.tile_pool(name="scr", bufs=1))
    bpB = ctx.enter_context(tc.tile_pool(name="bpB", bufs=1))
    w2pool = ctx.enter_context(tc.tile_pool(name="w2p", bufs=1))
    # wout + w2 loads: start as soon as scan-era SBUF frees; overlap A2A wait
    wout_stack = contextlib.ExitStack()
    woutpool = wout_stack.enter_context(tc.tile_pool(name="woutp", bufs=1))
    woutT_sb = woutpool.tile([128, DI // 128, DM], BF16)
    nc.sync.dma_start(woutT_sb[:, :, :], woutT.rearrange("(k p) m -> p k m", p=128))
    w2qT_sb = w2pool.tile([128, DFF // 128, DM], FP8)
    nc.scalar.dma_start(w2qT_sb[:, :, :], w2qT.rearrange("(k p) m -> p k m", p=128))

    x1_l, scl1_l, xqT_l, fqT_l, scl2_l = [], [], [], [], []

    # ---- out_proj + rmsnorm1 + quant1 ----
    with tc.tile_pool(name="oproj", bufs=1) as opool:
        nc.sync.dma_start(yfull[:, 1, :, :],
                          a2a_o[1].rearrange("(j p) t -> p j t", p=128))
        for tt in range(NTT):
            hps = psB.tile([128, DM], F32, tag="hps")
            for c in range(DM // 512):
                kk = 0
                for d in range(NDT):
                    for j in range(NCORES):
                        nc.tensor.matmul(
                            hps[:, c * 512:(c + 1) * 512],
                            yfull[:, d, j, tt * 128:(tt + 1) * 128],
                            woutT_sb[:, j * NDT + d, c * 512:(c + 1) * 512],
                            start=(kk == 0), stop=(kk == DI // 128 - 1))
                        kk += 1
            s = scr.tile([128, DM], F32, name=f"s{tt}", tag="sscr")
            nc.vector.tensor_tensor(s[:, :], x_tok_sb[:, tt, :], hps[:, :], op=OP.add)
            sq = scr.tile([128, DM], F32, name=f"sq{tt}", tag="sqscr")
            ssum = bpB.tile([128, 1], F32, name=f"ssum{tt}", tag="ssum")
            nc.scalar.activation(sq[:, :], s[:, :], AF.Square, accum_out=ssum[:, 0:1])
            v = bpB.tile([128, 1], F32, name=f"v{tt}", tag=f"v{tt}")
            nc.vector.tensor_scalar(v[:, :], ssum[:, :], 1.0 / DM, EPS,
                                    op0=OP.mult, op1=OP.add)
            nc.scalar.activation(v[:, :], v[:, :], AF.Sqrt)
            nc.vector.reciprocal(v[:, :], v[:, :])
            x1 = bpB.tile([128, DM], F32, name=f"x1_{tt}", tag=f"x1_{tt}")
            nc.vector.scalar_tensor_tensor(x1[:, :], s[:, :], v[:, 0:1],
                                           n1w_rep[:, :], op0=OP.mult, op1=OP.mult)
            x1_l.append(x1)
            if dbg and tt == 0:
                nc.sync.dma_start(dbg_t["dbg_x1"][:, :], x1[:, :])
            amax = bpB.tile([128, 1], F32, name=f"am{tt}", tag="am")
            nc.vector.tensor_reduce(amax[:, :], x1[:, :], axis=mybir.AxisListType.X,
                                    op=OP.max, apply_absolute_value=True)
            nc.vector.tensor_scalar(amax[:, :], amax[:, :], 1e-5, None, op0=OP.max)
            sc = bpB.tile([128, 1], F32, name=f"sc{tt}", tag="sc")
            nc.vector.reciprocal(sc[:, :], amax[:, :])
            nc.vector.tensor_scalar(sc[:, :], sc[:, :], 127.0, None, op0=OP.mult)
            scl1 = bpB.tile([128, 1], F32, name=f"scl1_{tt}", tag=f"scl1_{tt}")
            nc.vector.tensor_scalar(scl1[:, :], amax[:, :], g1 / 127.0, None,
                                    op0=OP.mult)
            scl1_l.append(scl1)
            q = scr.tile([128, DM], BF16, name=f"q{tt}", tag="qscr")
            nc.vector.tensor_scalar(q[:, :], x1[:, :], sc[:, 0:1], None, op0=OP.mult)
            nc.vector.tensor_scalar(q[:, :], q[:, :], MAGIC_BF, MAGIC_BF,
                                    op0=OP.add, op1=OP.subtract)
            xq = scr.tile([128, DM], BF16, name=f"xq{tt}", tag="xqscr")
            nc.vector.tensor_scalar(xq[:, :], q[:, :], 127.0, -128.0,
                                    op0=OP.min, op1=OP.max)
            # transpose via PE (8 x [128,128])
            xqT = bpB.tile([128, DM // 128, 128], BF16, name=f"xqT{tt}",
                           tag=f"xqT{tt}")
            for k in range(DM // 128):
                tp = psT.tile([128, 128], BF16, tag="tp")
                nc.tensor.transpose(tp[:, :], xq[:, k * 128:(k + 1) * 128],
                                    ident_bf[:, :])
                nc.scalar.copy(xqT[:, k, :], tp[:, :])
            xqT_l.append(xqT)
    wout_stack.close()

    # ---- FFN mm1 + gelu + quant2 ----
    with tc.tile_pool(name="ffn1", bufs=1) as f1pool:
        for tt in range(NTT):
            f_sb = f1pool.tile([128, DFF], BF16, name=f"f{tt}", tag=f"f{tt}")
            for jc in range(DFF // 512):
                fps = psB.tile([128, 512], F32, tag="mmps")
                for k in range(DM // 128):
                    nc.tensor.matmul(
                        fps[:, :], xqT_l[tt][:, k, :],
                        w1qT_sb[:, k, jc * 512:(jc + 1) * 512],
                        start=(k == 0), stop=(k == DM // 128 - 1))
                nc.scalar.activation(f_sb[:, jc * 512:(jc + 1) * 512], fps[:, :],
                                     AF.Gelu_apprx_tanh, scale=scl1_l[tt][:, 0:1])
            if dbg and tt == 0:
                f32dbg = f1pool.tile([128, DFF], F32, name="f32dbg", tag="f32dbg")
                nc.vector.tensor_copy(f32dbg[:, :], f_sb[:, :])
                nc.sync.dma_start(dbg_t["dbg_f"][:, :], f32dbg[:, :])
            amax2 = bpB.tile([128, 1], F32, name=f"am2{tt}", tag="am2")
            nc.vector.tensor_reduce(amax2[:, :], f_sb[:, :],
                                    axis=mybir.AxisListType.X,
                                    op=OP.max, apply_absolute_value=True)
            nc.vector.tensor_scalar(amax2[:, :], amax2[:, :], 1e-5, None, op0=OP.max)
            sc2 = bpB.tile([128, 1], F32, name=f"sc2{tt}", tag="sc2")
            nc.vector.reciprocal(sc2[:, :], amax2[:, :])
            nc.vector.tensor_scalar(sc2[:, :], sc2[:, :], 127.0, None, op0=OP.mult)
            scl2 = bpB.tile([128, 1], F32, name=f"scl2_{tt}", tag=f"scl2_{tt}")
            nc.vector.tensor_scalar(scl2[:, :], amax2[:, :], g2 / 127.0, None,
                                    op0=OP.mult)
            scl2_l.append(scl2)
            q2 = f1pool.tile([128, DFF], BF16, name=f"q2{tt}", tag="q2scr")
            nc.vector.tensor_scalar(q2[:, :], f_sb[:, :], sc2[:, 0:1], None,
                                    op0=OP.mult)
            nc.vector.tensor_scalar(q2[:, :], q2[:, :], MAGIC_BF, MAGIC_BF,
                                    op0=OP.add, op1=OP.subtract)
            fq = f1pool.tile([128, DFF], BF16, name=f"fq{tt}", tag="fqscr")
            nc.vector.tensor_scalar(fq[:, :], q2[:, :], 127.0, -128.0,
                                    op0=OP.min, op1=OP.max)
            fqT = bpB.tile([128, DFF // 128, 128], BF16, name=f"fqT{tt}",
                           tag=f"fqT{tt}")
            for k in range(DFF // 128):
                tp = psT.tile([128, 128], BF16, tag="tp")
                nc.tensor.transpose(tp[:, :], fq[:, k * 128:(k + 1) * 128],
                                    ident_bf[:, :])
                nc.scalar.copy(fqT[:, k, :], tp[:, :])
            fqT_l.append(fqT)

    # ---- FFN mm2 + residual + rmsnorm2 ----
    with tc.tile_pool(name="ffn2", bufs=1) as f2pool:
        for tt in range(NTT):
            o2 = f2pool.tile([128, DM], F32, name=f"o2{tt}", tag=f"o2{tt}")
            for mc in range(DM // 512):
                ops_ = psB.tile([128, 512], F32, tag="mmps")
                for k in range(DFF // 128):
                    nc.tensor.matmul(
                        ops_[:, :], fqT_l[tt][:, k, :],
                        w2qT_sb[:, k, mc * 512:(mc + 1) * 512],
                        start=(k == 0), stop=(k == DFF // 128 - 1))
                nc.vector.scalar_tensor_tensor(
                    o2[:, mc * 512:(mc + 1) * 512], ops_[:, :], scl2_l[tt][:, 0:1],
                    x1_l[tt][:, mc * 512:(mc + 1) * 512], op0=OP.mult, op1=OP.add)
            sq2 = f2pool.tile([128, DM], F32, name=f"sq2{tt}", tag="sq2scr")
            ssum2 = f2pool.tile([128, 1], F32, name=f"ssum2{tt}", tag="ssum2")
            nc.scalar.activation(sq2[:, :], o2[:, :], AF.Square,
                                 accum_out=ssum2[:, 0:1])
            v2 = f2pool.tile([128, 1], F32, name=f"v2{tt}", tag=f"v2{tt}")
            nc.vector.tensor_scalar(v2[:, :], ssum2[:, :], 1.0 / DM, EPS,
                                    op0=OP.mult, op1=OP.add)
            nc.scalar.activation(v2[:, :], v2[:, :], AF.Sqrt)
            nc.vector.reciprocal(v2[:, :], v2[:, :])
            ot = f2pool.tile([128, DM], F32, name=f"ot{tt}", tag="otscr")
            nc.vector.scalar_tensor_tensor(ot[:, :], o2[:, :], v2[:, 0:1],
                                           n2w_rep[:, :], op0=OP.mult, op1=OP.mult)
            nc.sync.dma_start(out_t[tt * 128:(tt + 1) * 128, :], ot[:, :])


def build_nc(g1, g2, dbg=False):
    from contextlib import ExitStack
    nc = bacc.Bacc("TRN2", target_bir_lowering=False, debug=False,
                   num_devices=NCORES)
    with ExitStack() as ctx:
        tc = ctx.enter_context(tile.TileContext(nc))
        _emit(nc, tc, ctx, g1, g2, dbg)
    nc.compile()
    return nc


def host_prep(inputs):
    bf = ml_dtypes.bfloat16
    f8 = ml_dtypes.float8_e4m3
    x = np.asarray(inputs["x"], np.float32)
    x2d = x.reshape(L, DM)
    w_in = np.asarray(inputs["w_in"], np.float32)
    conv_w = np.asarray(inputs["conv_w"], np.float32)
    conv_b = np.asarray(inputs["conv_b"], np.float32)
    w_xproj = np.asarray(inputs["w_xproj"], np.float32)
    w_dt = np.asarray(inputs["w_dt"], np.float32)
    b_dt = np.asarray(inputs["b_dt"], np.float32)
    A_log = np.asarray(inputs["A_log"], np.float32)
    Dp = np.asarray(inputs["Dp"], np.float32)
    w_out = np.asarray(inputs["w_out"], np.float32)
    n1 = np.asarray(inputs["norm1_w"], np.float32)
    n2 = np.asarray(inputs["norm2_w"], np.float32)
    w1 = np.asarray(inputs["ffn_w1"], np.float32)
    w2 = np.asarray(inputs["ffn_w2"], np.float32)
    b1 = np.asarray(inputs["ffn_b1"], np.float32)
    b2 = np.asarray(inputs["ffn_b2"], np.float32)
    assert np.all(b1 == 0.0) and np.all(b2 == 0.0), "nonzero ffn bias unsupported"

    g1 = float(np.maximum(np.mean(np.abs(w1), dtype=np.float32), 1e-5))
    g2 = float(np.maximum(np.mean(np.abs(w2), dtype=np.float32), 1e-5))
    w1q = np.clip(np.rint(w1 / g1), -1.0, 1.0).astype(np.float32)
    w2q = np.clip(np.rint(w2 / g2), -1.0, 1.0).astype(np.float32)

    xT_bf = np.ascontiguousarray(x2d.T).astype(bf)
    woutT_bf = np.ascontiguousarray(w_out.T).astype(bf)
    w1qT_f8 = np.ascontiguousarray(w1q.T).astype(f8)
    w2qT_f8 = np.ascontiguousarray(w2q.T).astype(f8)
    n1r = np.ascontiguousarray(n1.reshape(1, DM))
    n2r = np.ascontiguousarray(n2.reshape(1, DM))
    A = -np.exp(A_log)

    in_maps = []
    for c in range(NCORES):
        ch = slice(c * DIC, (c + 1) * DIC)
        w_sel = np.concatenate([w_in[c * DIC:(c + 1) * DIC],
                                w_in[DI + c * DIC:DI + (c + 1) * DIC]], axis=0)
        in_maps.append({
            "xT": xT_bf,
            "x_tok": np.ascontiguousarray(x2d[c * LT:(c + 1) * LT]),
            "winT": np.ascontiguousarray(w_sel.T).astype(bf),
            "convw": np.ascontiguousarray(conv_w[ch, 0, :]),
            "convb": np.ascontiguousarray(conv_b[ch].reshape(-1, 1)),
            "wxpT": np.ascontiguousarray(w_xproj[:, ch].T).astype(bf),
            "wdtT": np.ascontiguousarray(w_dt[ch, :].T).astype(bf),
            "bdt": np.ascontiguousarray(b_dt[ch].reshape(-1, 1)),
            "acol": np.ascontiguousarray(A[ch, :]),
            "dpv": np.ascontiguousarray(Dp[ch].reshape(-1, 1)),
            "woutT": woutT_bf,
            "n1w": n1r,
            "n2w": n2r,
            "w1qT": w1qT_f8,
            "w2qT": w2qT_f8,
        })
    return in_maps, g1, g2


def kernel(**inputs) -> np.ndarray:
    in_maps, g1, g2 = host_prep(inputs)
    key = (round(g1, 10), round(g2, 10))
    if key not in _NC_CACHE:
        _NC_CACHE[key] = build_nc(g1, g2)
    nc = _NC_CACHE[key]
    res = run_bass_kernel_spmd(nc, in_maps, core_ids=list(range(NCORES)))
    out = np.concatenate([res.results[c]["out"] for c in range(NCORES)], axis=0)
    return np.ascontiguousarray(out.reshape(1, L, DM).astype(np.float32))
